# revision 22
# baseline (speedup 1.0000x reference)
"""Trainium2 Bass kernel for a single transformer decoder layer.

Sharding: 8 cores = 4 batches x 2 head-groups (tensor parallel over heads for
attention; pairwise ReduceScatter; token-split FFN). All activations are kept
feature-major on device. bf16 everywhere on the matmul paths (fp32 PSUM
accumulation, fp32 residual/LN math). FFN W1 is fully SBUF-resident (prefetched
during attention); W2 streams per-dout. Softmax denominator broadcast uses a
tiny PE matmul so the gpsimd queue only runs the collectives.
"""

import sys

for _p in ("/opt/trn_rl_repo",):
    if _p not in sys.path:
        sys.path.insert(0, _p)

import numpy as np

import concourse.bass as bass
import concourse.mybir as mybir
import concourse.tile as tile
from concourse import bacc
from concourse.bass_utils import run_bass_kernel_spmd

# ---- problem constants (hardcoded per spec) ----
B, S, D = 4, 2048, 1024
H, DK, DV, DFF = 16, 64, 64, 4096
EPS = 1e-5
SCALE = 1.0 / 32.0  # 1/sqrt(D)

NCORES = 8
HL = H // 2          # heads per core (local)
NP = HL // 2         # head-pairs per core (4)
TLOC = S // 2        # tokens owned per core after reduce-scatter (1024)
DC = D // 128        # d-model chunks (8)
FC = DFF // 128      # dff chunks (32)
QB = S // 512        # query blocks of 512 (4)

F32 = mybir.dt.float32
F32R = mybir.dt.float32r
BF16 = mybir.dt.bfloat16

DEBUG = False
_COMPILED = None


def _build():
    nc = bacc.Bacc("TRN2", target_bir_lowering=False, debug=False,
                   num_devices=NCORES)

    xT_d = nc.dram_tensor("xT", [128, DC, S], BF16, kind="ExternalInput").ap()
    xTm_d = nc.dram_tensor("xTmine", [128, DC, TLOC], F32,
                           kind="ExternalInput").ap()
    wq_d = nc.dram_tensor("wq", [NP, 128, DC, 128], BF16,
                          kind="ExternalInput").ap()
    wk_d = nc.dram_tensor("wk", [NP, 128, DC, 128], BF16,
                          kind="ExternalInput").ap()
    wv_d = nc.dram_tensor("wv", [128, DC, 512], BF16, kind="ExternalInput").ap()
    wo_d = nc.dram_tensor("wo", [128, NP, DC, 128], BF16,
                          kind="ExternalInput").ap()
    w1_d = nc.dram_tensor("w1", [128, FC, DC, 128], BF16,
                          kind="ExternalInput").ap()
    w2_d = nc.dram_tensor("w2", [DC, 128, FC, 128], BF16,
                          kind="ExternalInput").ap()
    b1_d = nc.dram_tensor("b1s", [128, FC], F32, kind="ExternalInput").ap()
    b2_d = nc.dram_tensor("b2s", [128, DC], F32, kind="ExternalInput").ap()
    g1_d = nc.dram_tensor("g1s", [128, DC], F32, kind="ExternalInput").ap()
    e1_d = nc.dram_tensor("e1s", [128, DC], F32, kind="ExternalInput").ap()
    g2_d = nc.dram_tensor("g2s", [128, DC], F32, kind="ExternalInput").ap()
    e2_d = nc.dram_tensor("e2s", [128, DC], F32, kind="ExternalInput").ap()
    mk_d = nc.dram_tensor("maskt", [128, 128], BF16, kind="ExternalInput").ap()

    outT_d = nc.dram_tensor("outT", [128, DC, TLOC], F32,
                            kind="ExternalOutput").ap()
    dbg = None
    if DEBUG:
        dbg = {
            "dbg_ctx": nc.dram_tensor("dbg_ctx", [128, NP, S], F32,
                                      kind="ExternalOutput").ap(),
            "dbg_h1": nc.dram_tensor("dbg_h1", [128, DC, TLOC], F32,
                                     kind="ExternalOutput").ap(),
            "dbg_o2": nc.dram_tensor("dbg_o2", [128, DC, TLOC], F32,
                                     kind="ExternalOutput").ap(),
        }

    with tile.TileContext(nc) as tc:
        _emit(nc, tc, xT_d, xTm_d, wq_d, wk_d, wv_d, wo_d, w1_d, w2_d,
              b1_d, b2_d, g1_d, e1_d, g2_d, e2_d, mk_d, outT_d, dbg)
    nc.compile()
    return nc


def _emit(nc, tc, xT_d, xTm_d, wq_d, wk_d, wv_d, wo_d, w1_d, w2_d,
          b1_d, b2_d, g1_d, e1_d, g2_d, e2_d, mk_d, outT_d, dbg=None):
    AF = mybir.ActivationFunctionType

    with (
        tc.tile_pool(name="dram", bufs=1, space="DRAM") as dram,
        tc.tile_pool(name="const", bufs=1) as const,
        tc.tile_pool(name="pW1", bufs=1) as pW1,
    ):
        MASKT = const.tile([128, 128], BF16)
        nc.gpsimd.dma_start(MASKT[:], mk_d[:])
        onesf = const.tile([128, 1], F32)
        nc.vector.memset(onesf[:], 1.0)
        ones1 = const.tile([128, 1], F32R)
        nc.vector.tensor_copy(ones1[:], onesf[:])
        onesrf = const.tile([1, 128], F32)
        nc.vector.memset(onesrf[:], 1.0)
        onesr = const.tile([1, 128], F32R)
        nc.vector.tensor_copy(onesr[:], onesrf[:])
        epst = const.tile([1, 1], F32)
        nc.vector.memset(epst[:], EPS)
        g1t = const.tile([128, DC], F32)
        e1t = const.tile([128, DC], F32)
        g2t = const.tile([128, DC], F32)
        e2t = const.tile([128, DC], F32)
        b1t = const.tile([128, FC], F32)
        b2t = const.tile([128, DC], F32)
        for t_, d_ in ((g1t, g1_d), (e1t, e1_d), (g2t, g2_d), (e2t, e2_d),
                       (b1t, b1_d), (b2t, b2_d)):
            nc.sync.dma_start(t_[:], d_[:])

        # resident W1 (prefetched on the scalar engine's DMA queue; it is idle
        # until the first exp so the kicks go out immediately)
        w1s = pW1.tile([128, FC, DC, 128], BF16)
        for q4 in range(4):
            nc.scalar.dma_start(w1s[:, q4 * 8:(q4 + 1) * 8],
                                w1_d[:, q4 * 8:(q4 + 1) * 8])

        rs_in0 = dram.tile([2, D, 512], F32)
        rs_in1 = dram.tile([2, D, 512], F32)
        rs_out0 = dram.tile([D, 512], F32)
        rs_out1 = dram.tile([D, 512], F32)

        def layer_norm(src, dst, gt, et, work, psStat, statp, post=None):
            """feature-major LN over features of a [128, DC, 512] block.

            dst may be None; then per-dc outputs stream through a bounce tile
            given by dst_cb(dc, chunk_ap) -> None.
            """
            pmu = psStat.tile([1, 512], F32, tag="stat")
            psq = psStat.tile([1, 512], F32, tag="stat")
            for dc in range(DC):
                sq = work.tile([128, 512], F32R, tag="sq", bufs=2)
                nc.scalar.activation(sq[:], src[:, dc], AF.Square)
                nc.tensor.matmul(pmu[:], ones1[:], src[:, dc],
                                 start=(dc == 0), stop=(dc == DC - 1))
                nc.tensor.matmul(psq[:], ones1[:], sq[:],
                                 start=(dc == 0), stop=(dc == DC - 1))
            mu = statp.tile([1, 512], F32, tag="mu")
            ex2 = statp.tile([1, 512], F32, tag="ex2")
            nc.vector.tensor_scalar_mul(mu[:], pmu[:], 1.0 / D)
            nc.vector.tensor_scalar_mul(ex2[:], psq[:], 1.0 / D)
            var = statp.tile([1, 512], F32, tag="var")
            nc.vector.tensor_mul(var[:], mu[:], mu[:])
            nc.vector.tensor_sub(var[:], ex2[:], var[:])
            srt = statp.tile([1, 512], F32, tag="srt")
            nc.scalar.activation(srt[:], var[:], AF.Sqrt, bias=epst[:])
            rstd = statp.tile([1, 512], F32, tag="rstd")
            nc.vector.reciprocal(rstd[:], srt[:])
            nmr = statp.tile([1, 512], F32, tag="nmr")
            nc.vector.tensor_mul(nmr[:], mu[:], rstd[:])
            nc.vector.tensor_scalar_mul(nmr[:], nmr[:], -1.0)
            # broadcast rstd / -mu*rstd across partitions via a PE matmul
            rstf = statp.tile([1, 2, 512], F32R, tag="rbf")
            nc.vector.tensor_copy(rstf[:, 0], rstd[:])
            nc.vector.tensor_copy(rstf[:, 1], nmr[:])
            psb = psStat.tile([128, 2, 512], F32, tag="statb", bufs=1)
            nc.tensor.matmul(psb[:, 0], onesr[:], rstf[:, 0],
                             start=True, stop=True)
            nc.tensor.matmul(psb[:, 1], onesr[:], rstf[:, 1],
                             start=True, stop=True)
            for dc in range(DC):
                xh = work.tile([128, 512], F32, tag="xh", bufs=2)
                nc.vector.tensor_mul(xh[:], src[:, dc], psb[:, 0])
                nc.vector.tensor_add(xh[:], xh[:], psb[:, 1])
                ap = dst(dc)
                nc.scalar.activation(ap, xh[:], AF.Identity,
                                     bias=et[:, dc:dc + 1],
                                     scale=gt[:, dc:dc + 1])
                if post is not None:
                    post(dc, ap)

        # ============ attention: projections + qb-major attention ============
        with (
            tc.tile_pool(name="pQKT", bufs=1) as pQKT,
            tc.tile_pool(name="pV", bufs=1) as pV,
        ):
            QT = pQKT.tile([128, NP, S], BF16, tag="QT")          # 16KB
            KT = pQKT.tile([128, NP, S], BF16, tag="KT")          # 16KB
            V = pV.tile([128, S // 128, HL * 65], BF16, tag="V")  # 16.3KB
            nc.vector.tensor_copy(
                V[:].rearrange("p t (h c) -> p t h c", c=65)[:, :, :, 64:65],
                onesf[:, None, None, :].to_broadcast((128, S // 128, HL, 1)))

            with (
                tc.tile_pool(name="pX", bufs=1) as pX,
                tc.tile_pool(name="pWQK", bufs=2) as pWQK,
            ):
                X = pX.tile([128, DC, S], BF16, tag="X")          # 32KB
                for dc in range(DC):
                    nc.sync.dma_start(X[:, dc], xT_d[:, dc])

                # Q/K projections, dc-outer so PE starts on the first X chunk
                with tc.tile_pool(name="psP", bufs=8, space="PSUM") as psP:
                    for p in range(NP):
                        wqt = pWQK.tile([128, DC, 128], BF16, tag="wq")
                        wkt = pWQK.tile([128, DC, 128], BF16, tag="wk")
                        nc.gpsimd.dma_start(wqt[:], wq_d[p])
                        nc.gpsimd.dma_start(wkt[:], wk_d[p])
                        pqs = [psP.tile([128, 512], F32, tag="proj",
                                        name=f"pq_{i}") for i in range(8)]
                        for dc in range(DC):
                            for tb in range(QB):
                                nc.tensor.matmul(
                                    pqs[tb][:], wqt[:, dc],
                                    X[:, dc, tb * 512:(tb + 1) * 512],
                                    start=(dc == 0), stop=(dc == DC - 1))
                            for tb in range(QB):
                                nc.tensor.matmul(
                                    pqs[4 + tb][:], wkt[:, dc],
                                    X[:, dc, tb * 512:(tb + 1) * 512],
                                    start=(dc == 0), stop=(dc == DC - 1))
                        for tb in range(QB):
                            tsl = slice(tb * 512, (tb + 1) * 512)
                            nc.vector.tensor_copy(QT[:, p, tsl], pqs[tb][:])
                            nc.vector.tensor_copy(KT[:, p, tsl], pqs[4 + tb][:])

                # V projection (needs all of X)
                with (
                    tc.tile_pool(name="psV", bufs=3, space="PSUM") as psV,
                    tc.tile_pool(name="pWV", bufs=1) as pWV,
                ):
                    wvt = pWV.tile([128, DC, 512], BF16, tag="wv")
                    nc.gpsimd.dma_start(wvt[:], wv_d[:])
                    for tt in range(S // 128):
                        pv = psV.tile([128, 512], F32, tag="pv")
                        for dc in range(DC):
                            nc.tensor.matmul(pv[:],
                                             X[:, dc, tt * 128:(tt + 1) * 128],
                                             wvt[:, dc],
                                             start=(dc == 0), stop=(dc == DC - 1))
                        nc.vector.tensor_copy(
                            V[:, tt].rearrange("p (h c) -> p h c",
                                               c=65)[:, :, 0:64],
                            pv[:].rearrange("p (h c) -> p h c", c=64))

            # ---- attention, qb-outer; Wo + reduce-scatter interleaved ----
            with (
                tc.tile_pool(name="pCTX", bufs=1) as pCTX,
                tc.tile_pool(name="pWO", bufs=1) as pWO,
                tc.tile_pool(name="pE", bufs=3) as pE,
                tc.tile_pool(name="pAO", bufs=3) as pAO,
                tc.tile_pool(name="stB", bufs=2) as stB,
                tc.tile_pool(name="psS", bufs=2, space="PSUM") as psS,
                tc.tile_pool(name="psC", bufs=2, space="PSUM") as psC,
                tc.tile_pool(name="psW", bufs=2, space="PSUM") as psW,
            ):
                CTX = pCTX.tile([128, NP, S], BF16, tag="CTX")    # 16KB
                wot = pWO.tile([128, NP, DC, 128], BF16, tag="wo")
                nc.gpsimd.dma_start(wot[:], wo_d[:])

                def normalize(cts, p, qsl):
                    """softmax-normalize one p's two head columns into CTX."""
                    for row0, (cxt, den) in ((0, cts[0]), (64, cts[1])):
                        denb = psW.tile([128, 512], F32, tag="wo",
                                        name="denb")
                        nc.tensor.matmul(denb[0:64, :], onesr[:, 0:64],
                                         den[:], start=True, stop=True)
                        recb = stB.tile([64, 512], F32, tag="recb")
                        nc.vector.reciprocal(recb[:], denb[0:64, :])
                        nc.vector.tensor_mul(CTX[row0:row0 + 64, p, qsl],
                                             cxt[0:64, :], recb[:])

                for qb in range(QB):
                    qsl = slice(qb * 512, (qb + 1) * 512)
                    nkc = 4 * (qb + 1)
                    pending = None
                    for p in range(NP):
                        ctxA = psC.tile([65, 512], F32, tag="ctx")
                        ctxB = psC.tile([65, 512], F32, tag="ctx")

                        def emit_ctx(kc, eAB, off):
                            st, sp = (kc == 0), (kc == nkc - 1)
                            nc.tensor.matmul(
                                ctxA[:, off:],
                                V[:, kc, 2 * p * 65:(2 * p + 1) * 65],
                                eAB[:, 0, off:], start=st, stop=sp)
                            nc.tensor.matmul(
                                ctxB[:, off:],
                                V[:, kc, (2 * p + 1) * 65:(2 * p + 2) * 65],
                                eAB[:, 1, off:], start=st, stop=sp)

                        # ctx matmuls lag the scores by one kc so the PE
                        # stream never waits on exp/mask of the same kc
                        prev = None
                        for kc in range(nkc):
                            ksl = slice(kc * 128, (kc + 1) * 128)
                            diag = kc >= 4 * qb
                            off = (kc - 4 * qb) * 128 if diag else 0
                            qtr = slice(qb * 512 + off, (qb + 1) * 512)
                            sAB = psS.tile([128, 2, 512], F32, tag="sc")
                            nc.tensor.matmul(sAB[:, 0, off:],
                                             KT[0:64, p, ksl],
                                             QT[0:64, p, qtr],
                                             start=True, stop=True)
                            nc.tensor.matmul(sAB[:, 1, off:],
                                             KT[64:128, p, ksl],
                                             QT[64:128, p, qtr],
                                             start=True, stop=True)
                            if prev is not None:
                                emit_ctx(*prev)
                            eAB = pE.tile([128, 2, 512], BF16, tag="E")
                            nc.scalar.activation(eAB[:, :, off:],
                                                 sAB[:, :, off:],
                                                 AF.Exp, scale=SCALE)
                            if diag:
                                nc.vector.tensor_mul(
                                    eAB[:, :, off:off + 128],
                                    eAB[:, :, off:off + 128],
                                    MASKT[:, None, :].to_broadcast(
                                        (128, 2, 128)))
                            prev = (kc, eAB, off)
                        emit_ctx(*prev)
                        dens = []
                        for i, cxt in enumerate((ctxA, ctxB)):
                            den = stB.tile([1, 512], F32R, tag="den",
                                           name=f"den{i}", bufs=4)
                            nc.vector.tensor_copy(den[:], cxt[64:65, :])
                            dens.append((cxt, den))
                        # normalize lags one p so its PE matmuls never stall
                        if pending is not None:
                            normalize(pending[0], pending[1], qsl)
                        pending = (dens, p)
                    normalize(pending[0], pending[1], qsl)
                    # Wo partial for this token block
                    rsdst = rs_in0 if qb < 2 else rs_in1
                    for dout in range(DC):
                        po = psW.tile([128, 512], F32, tag="wo")
                        for p in range(NP):
                            nc.tensor.matmul(po[:], wot[:, p, dout],
                                             CTX[:, p, qsl],
                                             start=(p == 0), stop=(p == NP - 1))
                        ao = pAO.tile([128, 512], F32, tag="ao")
                        nc.vector.tensor_copy(ao[:], po[:])
                        nc.sync.dma_start(
                            rsdst[qb % 2, dout * 128:(dout + 1) * 128, :],
                            ao[:])
                    if qb == 1:
                        nc.gpsimd.collective_compute(
                            "ReduceScatter", mybir.AluOpType.add,
                            replica_groups=[[0, 1], [2, 3], [4, 5], [6, 7]],
                            ins=[rs_in0.opt()], outs=[rs_out0.opt()])
                if dbg is not None:
                    nc.gpsimd.dma_start(dbg["dbg_ctx"][:], CTX[:])

        # ======== LN1 halves + FFN ========
        with (
            tc.tile_pool(name="pH1", bufs=1) as pH1,
            tc.tile_pool(name="pAOr", bufs=1) as pAOr,
            tc.tile_pool(name="pLN", bufs=1) as pLN,
            tc.tile_pool(name="stDE", bufs=1) as stDE,
            tc.tile_pool(name="psD", bufs=2, space="PSUM") as psD,
        ):
            H1 = [pH1.tile([128, DC, 512], BF16, tag=f"H1_{h}",
                           name=f"H1_{h}") for h in range(2)]

            def d_half(h, rso):
                aor = pAOr.tile([128, DC, 512], F32R, tag="AOr",
                                name=f"AOr{h}")
                nc.gpsimd.dma_start(
                    aor[:], rso.rearrange("(dc p) t -> p dc t", p=128))
                xm = pAOr.tile([128, DC, 512], F32, tag="XM",
                               name=f"XMt{h}")
                nc.sync.dma_start(xm[:], xTm_d[:, :, h * 512:(h + 1) * 512])
                nc.vector.tensor_add(aor[:], aor[:], xm[:])
                layer_norm(aor[:], lambda dc: H1[h][:, dc], g1t, e1t,
                           pLN, psD, stDE)
                if dbg is not None:
                    nc.gpsimd.dma_start(
                        dbg["dbg_h1"][:, :, h * 512:(h + 1) * 512],
                        H1[h][:])

            d_half(0, rs_out0)
            # second reduce-scatter (gpsimd blocks on collectives; it has no
            # other work left)
            nc.gpsimd.collective_compute(
                "ReduceScatter", mybir.AluOpType.add,
                replica_groups=[[0, 1], [2, 3], [4, 5], [6, 7]],
                ins=[rs_in1.opt()], outs=[rs_out1.opt()])

            # ======== FFN: W1 (resident weights) + W2 (streamed) ========
            with (
                tc.tile_pool(name="pFF", bufs=1) as pFF,
                tc.tile_pool(name="pO2", bufs=1) as pO2,
                tc.tile_pool(name="pW2q", bufs=2) as pW2q,
                tc.tile_pool(name="psF", bufs=2, space="PSUM") as psF,
                tc.tile_pool(name="psG", bufs=2, space="PSUM") as psG,
            ):
                def ffn_w1(th):
                    FFt = pFF.tile([128, FC, 512], BF16, tag="FF",
                                   name=f"FFt{th}")  # 32KB
                    for fc in range(FC):
                        pf = psF.tile([128, 512], F32, tag="ff")
                        for dc in range(DC):
                            nc.tensor.matmul(
                                pf[:], w1s[:, fc, dc], H1[th][:, dc],
                                start=(dc == 0), stop=(dc == DC - 1))
                        nc.scalar.activation(FFt[:, fc], pf[:], AF.Relu,
                                             bias=b1t[:, fc:fc + 1])
                    return FFt

                def ffn_w2(th, FFt):
                    O2 = pO2.tile([128, DC, 512], F32R, tag="O2",
                                  name=f"O2_{th}")
                    for dout in range(DC):
                        w2t = pW2q.tile([128, FC, 128], BF16, tag="w2")
                        nc.sync.dma_start(w2t[:], w2_d[dout])
                        po2 = psG.tile([128, 512], F32, tag="o2")
                        for fc in range(FC):
                            nc.tensor.matmul(po2[:], w2t[:, fc], FFt[:, fc],
                                             start=(fc == 0),
                                             stop=(fc == FC - 1))
                        # O2[dout] = (po2 + b2[dout]) + h1[dout]
                        nc.vector.scalar_tensor_tensor(
                            O2[:, dout], po2[:], b2t[:, dout:dout + 1],
                            H1[th][:, dout],
                            mybir.AluOpType.add, mybir.AluOpType.add)
                    if dbg is not None:
                        nc.gpsimd.dma_start(
                            dbg["dbg_o2"][:, :, th * 512:(th + 1) * 512],
                            O2[:])
                    def ot_dst(dc):
                        otc = pFF.tile([128, 512], F32, tag="ot", bufs=2,
                                       name="otc")
                        return otc[:]

                    def ot_post(dc, ap):
                        nc.sync.dma_start(
                            outT_d[:, dc, th * 512:(th + 1) * 512], ap)

                    layer_norm(O2[:], ot_dst, g2t, e2t, pLN, psD, stDE,
                               post=ot_post)

                ff0 = ffn_w1(0)
                d_half(1, rs_out1)
                ffn_w2(0, ff0)
                ff1 = ffn_w1(1)
                ffn_w2(1, ff1)


def _pack_inputs(x, Wq, Wk, Wv, Wo, ln1_g, ln1_b, W1, b1, W2, b2, ln2_g, ln2_b):
    """Build the 8 per-core input maps (all host-side numpy)."""
    import ml_dtypes
    bf = ml_dtypes.bfloat16
    f = np.float32
    x = np.asarray(x, f)
    Wq = np.asarray(Wq, f); Wk = np.asarray(Wk, f); Wv = np.asarray(Wv, f)
    Wo = np.asarray(Wo, f); W1 = np.asarray(W1, f); W2 = np.asarray(W2, f)
    in_maps = []
    w1p = np.ascontiguousarray(
        W1.reshape(DC, 128, FC, 128).transpose(1, 2, 0, 3)).astype(bf)
    w2p = np.ascontiguousarray(
        W2.reshape(FC, 128, DC, 128).transpose(2, 1, 0, 3)).astype(bf)
    b1s = np.ascontiguousarray(np.asarray(b1, f).reshape(FC, 128).T)
    b2s = np.ascontiguousarray(np.asarray(b2, f).reshape(DC, 128).T)
    g1s = np.ascontiguousarray(np.asarray(ln1_g, f).reshape(DC, 128).T)
    e1s = np.ascontiguousarray(np.asarray(ln1_b, f).reshape(DC, 128).T)
    g2s = np.ascontiguousarray(np.asarray(ln2_g, f).reshape(DC, 128).T)
    e2s = np.ascontiguousarray(np.asarray(ln2_b, f).reshape(DC, 128).T)
    kk = np.arange(128)[:, None]
    qq = np.arange(128)[None, :]
    maskt = np.ascontiguousarray((kk <= qq).astype(f)).astype(bf)

    for c in range(NCORES):
        b, j = c // 2, c % 2
        hb = j * HL
        xT = np.ascontiguousarray(
            x[b].T.reshape(DC, 128, S).transpose(1, 0, 2)).astype(bf)
        # owned token blocks: {j, j+2} of four 512-blocks
        xm = np.concatenate(
            [x[b, j * 512:(j + 1) * 512],
             x[b, (j + 2) * 512:(j + 3) * 512]]).T  # [D, TLOC]
        xTm = np.ascontiguousarray(
            xm.reshape(DC, 128, TLOC).transpose(1, 0, 2))
        wq = np.stack([np.concatenate([Wq[hb + 2 * p], Wq[hb + 2 * p + 1]], 1)
                       for p in range(NP)])  # [NP, D, 128]
        wq = np.ascontiguousarray(
            wq.reshape(NP, DC, 128, 128).transpose(0, 2, 1, 3)).astype(bf)
        wk = np.stack([np.concatenate([Wk[hb + 2 * p], Wk[hb + 2 * p + 1]], 1)
                       for p in range(NP)])
        wk = np.ascontiguousarray(
            wk.reshape(NP, DC, 128, 128).transpose(0, 2, 1, 3)).astype(bf)
        wv = np.concatenate([Wv[hb + i] for i in range(HL)], 1)  # [D, 512]
        wv = np.ascontiguousarray(
            wv.reshape(DC, 128, 512).transpose(1, 0, 2)).astype(bf)
        wo = Wo[j * 512:(j + 1) * 512]  # [512, D]
        wo = np.ascontiguousarray(
            wo.reshape(NP, 128, DC, 128).transpose(1, 0, 2, 3)).astype(bf)
        in_maps.append({
            "xT": xT, "xTmine": xTm, "wq": wq, "wk": wk, "wv": wv, "wo": wo,
            "w1": w1p, "w2": w2p, "b1s": b1s, "b2s": b2s,
            "g1s": g1s, "e1s": e1s, "g2s": g2s, "e2s": e2s, "maskt": maskt,
        })
    return in_maps


def get_compiled():
    global _COMPILED
    if _COMPILED is None:
        _COMPILED = _build()
    return _COMPILED


def kernel(x, Wq, Wk, Wv, Wo, ln1_g, ln1_b, W1, b1, W2, b2, ln2_g, ln2_b,
           _trace=False):
    nc = get_compiled()
    in_maps = _pack_inputs(x, Wq, Wk, Wv, Wo, ln1_g, ln1_b, W1, b1, W2, b2,
                           ln2_g, ln2_b)
    res = run_bass_kernel_spmd(nc, in_maps, core_ids=list(range(NCORES)),
                               trace=_trace)
    out = np.zeros((B, S, D), np.float32)
    for c in range(NCORES):
        b, j = c // 2, c % 2
        o = res.results[c]["outT"]  # [128, DC, TLOC]
        o = np.asarray(o, np.float32).transpose(1, 0, 2).reshape(D, TLOC)
        out[b, j * 512:(j + 1) * 512, :] = o[:, 0:512].T
        out[b, (j + 2) * 512:(j + 3) * 512, :] = o[:, 512:1024].T
    kernel.last_result = res
    return out


# revision 34
# speedup vs baseline: 1.0244x; 1.0244x over previous
"""Trainium2 Bass kernel for a single transformer decoder layer.

Sharding: 8 cores = 4 batches x 2 head-groups (tensor parallel over heads for
attention; pairwise ReduceScatter; token-split FFN). All activations are kept
feature-major on device. bf16 everywhere on the matmul paths (fp32 PSUM
accumulation, fp32 residual/LN math). FFN W1 is fully SBUF-resident (prefetched
during attention); W2 streams per-dout. Softmax denominator broadcast uses a
tiny PE matmul so the gpsimd queue only runs the collectives.
"""

import sys

for _p in ("/opt/trn_rl_repo",):
    if _p not in sys.path:
        sys.path.insert(0, _p)

import numpy as np

import concourse.bass as bass
import concourse.mybir as mybir
import concourse.tile as tile
from concourse import bacc
from concourse.bass_utils import run_bass_kernel_spmd

# ---- problem constants (hardcoded per spec) ----
B, S, D = 4, 2048, 1024
H, DK, DV, DFF = 16, 64, 64, 4096
EPS = 1e-5
SCALE = 1.0 / 32.0  # 1/sqrt(D)

NCORES = 8
HL = H // 2          # heads per core (local)
NP = HL // 2         # head-pairs per core (4)
TLOC = S // 2        # tokens owned per core after reduce-scatter (1024)
DC = D // 128        # d-model chunks (8)
FC = DFF // 128      # dff chunks (32)
QB = S // 512        # query blocks of 512 (4)

F32 = mybir.dt.float32
F32R = mybir.dt.float32r
BF16 = mybir.dt.bfloat16

DEBUG = False
_COMPILED = None


def _build():
    nc = bacc.Bacc("TRN2", target_bir_lowering=False, debug=False,
                   num_devices=NCORES)

    xT_d = nc.dram_tensor("xT", [128, DC, S], BF16, kind="ExternalInput").ap()
    xTm_d = nc.dram_tensor("xTmine", [128, DC, TLOC], F32,
                           kind="ExternalInput").ap()
    wq_d = nc.dram_tensor("wq", [NP, 128, DC, 128], BF16,
                          kind="ExternalInput").ap()
    wk_d = nc.dram_tensor("wk", [NP, 128, DC, 128], BF16,
                          kind="ExternalInput").ap()
    wv_d = nc.dram_tensor("wv", [128, DC, 512], BF16, kind="ExternalInput").ap()
    wo_d = nc.dram_tensor("wo", [128, NP, DC, 128], BF16,
                          kind="ExternalInput").ap()
    w1_d = nc.dram_tensor("w1", [128, FC, DC, 128], BF16,
                          kind="ExternalInput").ap()
    w2_d = nc.dram_tensor("w2", [DC, 128, FC, 128], BF16,
                          kind="ExternalInput").ap()
    b1_d = nc.dram_tensor("b1s", [128, FC], F32, kind="ExternalInput").ap()
    b2_d = nc.dram_tensor("b2s", [128, DC], F32, kind="ExternalInput").ap()
    g1_d = nc.dram_tensor("g1s", [128, DC], F32, kind="ExternalInput").ap()
    e1_d = nc.dram_tensor("e1s", [128, DC], F32, kind="ExternalInput").ap()
    g2_d = nc.dram_tensor("g2s", [128, DC], F32, kind="ExternalInput").ap()
    e2_d = nc.dram_tensor("e2s", [128, DC], F32, kind="ExternalInput").ap()
    mk_d = nc.dram_tensor("maskt", [128, 128], BF16, kind="ExternalInput").ap()

    outT_d = nc.dram_tensor("outT", [128, DC, TLOC], F32,
                            kind="ExternalOutput").ap()
    dbg = None
    if DEBUG:
        dbg = {
            "dbg_ctx": nc.dram_tensor("dbg_ctx", [128, NP, S], F32,
                                      kind="ExternalOutput").ap(),
            "dbg_h1": nc.dram_tensor("dbg_h1", [128, DC, TLOC], F32,
                                     kind="ExternalOutput").ap(),
            "dbg_o2": nc.dram_tensor("dbg_o2", [128, DC, TLOC], F32,
                                     kind="ExternalOutput").ap(),
        }

    with tile.TileContext(nc) as tc:
        _emit(nc, tc, xT_d, xTm_d, wq_d, wk_d, wv_d, wo_d, w1_d, w2_d,
              b1_d, b2_d, g1_d, e1_d, g2_d, e2_d, mk_d, outT_d, dbg)
    nc.compile()
    return nc


def _emit(nc, tc, xT_d, xTm_d, wq_d, wk_d, wv_d, wo_d, w1_d, w2_d,
          b1_d, b2_d, g1_d, e1_d, g2_d, e2_d, mk_d, outT_d, dbg=None):
    AF = mybir.ActivationFunctionType

    with (
        tc.tile_pool(name="dram", bufs=1, space="DRAM") as dram,
        tc.tile_pool(name="const", bufs=1) as const,
        tc.tile_pool(name="pW1", bufs=1) as pW1,
    ):
        MASKT = const.tile([128, 128], BF16)
        nc.gpsimd.dma_start(MASKT[:], mk_d[:])
        onesf = const.tile([128, 1], F32)
        nc.vector.memset(onesf[:], 1.0)
        ones1 = const.tile([128, 1], F32R)
        nc.vector.tensor_copy(ones1[:], onesf[:])
        onesrf = const.tile([1, 128], F32)
        nc.vector.memset(onesrf[:], 1.0)
        onesr = const.tile([1, 128], F32R)
        nc.vector.tensor_copy(onesr[:], onesrf[:])
        epst = const.tile([1, 1], F32)
        nc.vector.memset(epst[:], EPS)
        g1t = const.tile([128, DC], F32)
        e1t = const.tile([128, DC], F32)
        g2t = const.tile([128, DC], F32)
        e2t = const.tile([128, DC], F32)
        b1t = const.tile([128, FC], F32)
        b2t = const.tile([128, DC], F32)
        for t_, d_ in ((g1t, g1_d), (e1t, e1_d), (g2t, g2_d), (e2t, e2_d),
                       (b1t, b1_d), (b2t, b2_d)):
            nc.sync.dma_start(t_[:], d_[:])

        # resident W1 (prefetched on the scalar engine's DMA queue; it is idle
        # until the first exp so the kicks go out immediately)
        w1s = pW1.tile([128, FC, DC, 128], BF16)
        for q4 in range(4):
            nc.scalar.dma_start(w1s[:, q4 * 8:(q4 + 1) * 8],
                                w1_d[:, q4 * 8:(q4 + 1) * 8])

        F16 = mybir.dt.float16
        rs_in0 = dram.tile([2, D, 512], F16)
        rs_in1 = dram.tile([2, D, 512], F16)
        rs_out0 = dram.tile([D, 512], F16)
        rs_out1 = dram.tile([D, 512], F16)
        dnd = dram.tile([NP, 2, 512], F32)  # softmax denominator bounce

        def layer_norm(src, dst, gt, et, work, psStat, statp, post=None):
            """feature-major LN over features of a [128, DC, 512] block.

            dst may be None; then per-dc outputs stream through a bounce tile
            given by dst_cb(dc, chunk_ap) -> None.
            """
            pmu = psStat.tile([1, 512], F32, tag="stat")
            psq = psStat.tile([1, 512], F32, tag="stat")
            for dc in range(DC):
                sq = work.tile([128, 512], F32R, tag="sq", bufs=2)
                nc.scalar.activation(sq[:], src[:, dc], AF.Square)
                nc.tensor.matmul(pmu[:], ones1[:], src[:, dc],
                                 start=(dc == 0), stop=(dc == DC - 1))
                nc.tensor.matmul(psq[:], ones1[:], sq[:],
                                 start=(dc == 0), stop=(dc == DC - 1))
            mu = statp.tile([1, 512], F32, tag="mu")
            ex2 = statp.tile([1, 512], F32, tag="ex2")
            nc.vector.tensor_scalar_mul(mu[:], pmu[:], 1.0 / D)
            nc.vector.tensor_scalar_mul(ex2[:], psq[:], 1.0 / D)
            var = statp.tile([1, 512], F32, tag="var")
            nc.vector.tensor_mul(var[:], mu[:], mu[:])
            nc.vector.tensor_sub(var[:], ex2[:], var[:])
            srt = statp.tile([1, 512], F32, tag="srt")
            nc.scalar.activation(srt[:], var[:], AF.Sqrt, bias=epst[:])
            rstd = statp.tile([1, 512], F32, tag="rstd")
            nc.vector.reciprocal(rstd[:], srt[:])
            nmr = statp.tile([1, 512], F32, tag="nmr")
            nc.vector.tensor_mul(nmr[:], mu[:], rstd[:])
            nc.vector.tensor_scalar_mul(nmr[:], nmr[:], -1.0)
            # broadcast rstd / -mu*rstd across partitions via a PE matmul
            rstf = statp.tile([1, 2, 512], F32R, tag="rbf")
            nc.vector.tensor_copy(rstf[:, 0], rstd[:])
            nc.vector.tensor_copy(rstf[:, 1], nmr[:])
            psb = psStat.tile([128, 2, 512], F32, tag="statb", bufs=1)
            nc.tensor.matmul(psb[:, 0], onesr[:], rstf[:, 0],
                             start=True, stop=True)
            nc.tensor.matmul(psb[:, 1], onesr[:], rstf[:, 1],
                             start=True, stop=True)
            for dc in range(DC):
                xh = work.tile([128, 512], F32, tag="xh", bufs=2)
                nc.vector.tensor_mul(xh[:], src[:, dc], psb[:, 0])
                nc.vector.tensor_add(xh[:], xh[:], psb[:, 1])
                ap = dst(dc)
                nc.scalar.activation(ap, xh[:], AF.Identity,
                                     bias=et[:, dc:dc + 1],
                                     scale=gt[:, dc:dc + 1])
                if post is not None:
                    post(dc, ap)

        # ============ attention: projections + qb-major attention ============
        with (
            tc.tile_pool(name="pQKT", bufs=1) as pQKT,
            tc.tile_pool(name="pV", bufs=1) as pV,
        ):
            QT = pQKT.tile([128, NP, S], BF16, tag="QT")          # 16KB
            KT = pQKT.tile([128, NP, S], BF16, tag="KT")          # 16KB
            V = pV.tile([128, S // 128, HL * 65], BF16, tag="V")  # 16.3KB
            nc.vector.tensor_copy(
                V[:].rearrange("p t (h c) -> p t h c", c=65)[:, :, :, 64:65],
                onesf[:, None, None, :].to_broadcast((128, S // 128, HL, 1)))

            with (
                tc.tile_pool(name="pX", bufs=1) as pX,
                tc.tile_pool(name="pWQK", bufs=2) as pWQK,
            ):
                X = pX.tile([128, DC, S], BF16, tag="X")          # 32KB
                for dc in range(DC):
                    nc.sync.dma_start(X[:, dc], xT_d[:, dc])

                # Q/K projections, dc-outer so PE starts on the first X chunk
                with tc.tile_pool(name="psP", bufs=8, space="PSUM") as psP:
                    for p in range(NP):
                        wqt = pWQK.tile([128, DC, 128], BF16, tag="wq")
                        wkt = pWQK.tile([128, DC, 128], BF16, tag="wk")
                        nc.gpsimd.dma_start(wqt[:], wq_d[p])
                        nc.gpsimd.dma_start(wkt[:], wk_d[p])
                        pqs = [psP.tile([128, 512], F32, tag="proj",
                                        name=f"pq_{i}") for i in range(8)]
                        for dc in range(DC):
                            for tb in range(QB):
                                nc.tensor.matmul(
                                    pqs[tb][:], wqt[:, dc],
                                    X[:, dc, tb * 512:(tb + 1) * 512],
                                    start=(dc == 0), stop=(dc == DC - 1))
                            for tb in range(QB):
                                nc.tensor.matmul(
                                    pqs[4 + tb][:], wkt[:, dc],
                                    X[:, dc, tb * 512:(tb + 1) * 512],
                                    start=(dc == 0), stop=(dc == DC - 1))
                        for tb in range(QB):
                            tsl = slice(tb * 512, (tb + 1) * 512)
                            nc.vector.tensor_copy(QT[:, p, tsl], pqs[tb][:])
                            nc.vector.tensor_copy(KT[:, p, tsl], pqs[4 + tb][:])

                # V projection (needs all of X)
                with (
                    tc.tile_pool(name="psV", bufs=3, space="PSUM") as psV,
                    tc.tile_pool(name="pWV", bufs=1) as pWV,
                ):
                    wvt = pWV.tile([128, DC, 512], BF16, tag="wv")
                    nc.gpsimd.dma_start(wvt[:], wv_d[:])
                    for tt in range(S // 128):
                        pv = psV.tile([128, 512], F32, tag="pv")
                        for dc in range(DC):
                            nc.tensor.matmul(pv[:],
                                             X[:, dc, tt * 128:(tt + 1) * 128],
                                             wvt[:, dc],
                                             start=(dc == 0), stop=(dc == DC - 1))
                        nc.vector.tensor_copy(
                            V[:, tt].rearrange("p (h c) -> p h c",
                                               c=65)[:, :, 0:64],
                            pv[:].rearrange("p (h c) -> p h c", c=64))

            # ---- attention, qb-outer; Wo + reduce-scatter interleaved ----
            with (
                tc.tile_pool(name="pCTX", bufs=1) as pCTX,
                tc.tile_pool(name="pWO", bufs=1) as pWO,
                tc.tile_pool(name="pE", bufs=3) as pE,
                tc.tile_pool(name="pAO", bufs=3) as pAO,
                tc.tile_pool(name="stB", bufs=2) as stB,
                tc.tile_pool(name="psS", bufs=2, space="PSUM") as psS,
                tc.tile_pool(name="psC", bufs=4, space="PSUM") as psC,
            ):
                CTX = pCTX.tile([128, NP, S], BF16, tag="CTX")    # 16KB
                wot = pWO.tile([128, NP, DC, 128], BF16, tag="wo")
                nc.gpsimd.dma_start(wot[:], wo_d[:])

                def normalize(cts, p, qsl):
                    """softmax-normalize one p's two head columns into CTX.

                    The [1,512] denominator is broadcast to 64 partitions via
                    a DRAM round-trip DMA: no PSUM bank, no compute engine.
                    """
                    for i, (row0, (cxt, den)) in enumerate(
                            ((0, cts[0]), (64, cts[1]))):
                        denB = stB.tile([64, 512], F32, tag="denB", bufs=2)
                        nc.sync.dma_start(dnd[p, i][None, :], den[:])
                        nc.sync.dma_start(
                            denB[:],
                            dnd[p, i][None, :].to_broadcast((64, 512)))
                        recb = stB.tile([64, 512], F32, tag="recb")
                        nc.vector.reciprocal(recb[:], denB[:])
                        nc.vector.tensor_mul(CTX[row0:row0 + 64, p, qsl],
                                             cxt[0:64, :], recb[:])

                for qb in range(QB):
                    qsl = slice(qb * 512, (qb + 1) * 512)
                    nkc = 4 * (qb + 1)
                    pending = []
                    for p in range(NP):
                        ctxA = psC.tile([65, 512], F32, tag="ctx")
                        ctxB = psC.tile([65, 512], F32, tag="ctx")

                        def emit_ctx(kc, eAB, off):
                            st, sp = (kc == 0), (kc == nkc - 1)
                            nc.tensor.matmul(
                                ctxA[:, off:],
                                V[:, kc, 2 * p * 65:(2 * p + 1) * 65],
                                eAB[:, 0, off:], start=st, stop=sp)
                            nc.tensor.matmul(
                                ctxB[:, off:],
                                V[:, kc, (2 * p + 1) * 65:(2 * p + 2) * 65],
                                eAB[:, 1, off:], start=st, stop=sp)

                        # ctx matmuls lag the scores by one kc so the PE
                        # stream never waits on exp/mask of the same kc
                        prev = None
                        first = True
                        for kc in range(nkc):
                            ksl = slice(kc * 128, (kc + 1) * 128)
                            diag = kc >= 4 * qb
                            off = (kc - 4 * qb) * 128 if diag else 0
                            qtr = slice(qb * 512 + off, (qb + 1) * 512)
                            sAB = psS.tile([128, 2, 512], F32, tag="sc")
                            nc.tensor.matmul(sAB[:, 0, off:],
                                             KT[0:64, p, ksl],
                                             QT[0:64, p, qtr],
                                             start=True, stop=True)
                            nc.tensor.matmul(sAB[:, 1, off:],
                                             KT[64:128, p, ksl],
                                             QT[64:128, p, qtr],
                                             start=True, stop=True)
                            if prev is not None:
                                emit_ctx(*prev)
                            elif first and pending:
                                # normalize of p-1 emitted early inside p's
                                # loop so its DVE chain overlaps p's compute
                                first = False
                                pn = pending.pop(0)
                                normalize(pn[0], pn[1], qsl)
                            eAB = pE.tile([128, 2, 512], BF16, tag="E")
                            nc.scalar.activation(eAB[:, :, off:],
                                                 sAB[:, :, off:],
                                                 AF.Exp, scale=SCALE)
                            if diag:
                                nc.vector.tensor_mul(
                                    eAB[:, :, off:off + 128],
                                    eAB[:, :, off:off + 128],
                                    MASKT[:, None, :].to_broadcast(
                                        (128, 2, 128)))
                            prev = (kc, eAB, off)
                        emit_ctx(*prev)
                        dens = []
                        for i, cxt in enumerate((ctxA, ctxB)):
                            den = stB.tile([1, 512], F32, tag="den",
                                           name=f"den{i}", bufs=4)
                            nc.vector.tensor_copy(den[:], cxt[64:65, :])
                            dens.append((cxt, den))
                        pending.append((dens, p))
                    for pn in pending:
                        normalize(pn[0], pn[1], qsl)
                    pending = []
                    # Wo partial for this token block
                    rsdst = rs_in0 if qb < 2 else rs_in1
                    for dout in range(DC):
                        po = psS.tile([128, 512], F32, tag="sc", name="po")
                        for p in range(NP):
                            nc.tensor.matmul(po[:], wot[:, p, dout],
                                             CTX[:, p, qsl],
                                             start=(p == 0), stop=(p == NP - 1))
                        ao = pAO.tile([128, 512], mybir.dt.float16, tag="ao")
                        nc.vector.tensor_copy(ao[:], po[:])
                        nc.sync.dma_start(
                            rsdst[qb % 2, dout * 128:(dout + 1) * 128, :],
                            ao[:])
                    if qb == 1:
                        nc.gpsimd.collective_compute(
                            "ReduceScatter", mybir.AluOpType.add,
                            replica_groups=[[0, 1], [2, 3], [4, 5], [6, 7]],
                            ins=[rs_in0.opt()], outs=[rs_out0.opt()])
                if dbg is not None:
                    nc.gpsimd.dma_start(dbg["dbg_ctx"][:], CTX[:])

        # ======== LN1 halves + FFN ========
        with (
            tc.tile_pool(name="pH1", bufs=1) as pH1,
            tc.tile_pool(name="pAOr", bufs=1) as pAOr,
            tc.tile_pool(name="pLN", bufs=1) as pLN,
            tc.tile_pool(name="stDE", bufs=1) as stDE,
            tc.tile_pool(name="psD", bufs=2, space="PSUM") as psD,
        ):
            H1 = [pH1.tile([128, DC, 512], BF16, tag=f"H1_{h}",
                           name=f"H1_{h}") for h in range(2)]

            def d_half(h, rso):
                aor = pAOr.tile([128, DC, 512], F32R, tag="AOr",
                                name=f"AOr{h}")
                nc.gpsimd.dma_start(
                    aor[:], rso.rearrange("(dc p) t -> p dc t", p=128))
                xm = pAOr.tile([128, DC, 512], F32, tag="XM",
                               name=f"XMt{h}")
                nc.sync.dma_start(xm[:], xTm_d[:, :, h * 512:(h + 1) * 512])
                nc.vector.tensor_add(aor[:], aor[:], xm[:])
                layer_norm(aor[:], lambda dc: H1[h][:, dc], g1t, e1t,
                           pLN, psD, stDE)
                if dbg is not None:
                    nc.gpsimd.dma_start(
                        dbg["dbg_h1"][:, :, h * 512:(h + 1) * 512],
                        H1[h][:])

            d_half(0, rs_out0)
            # second reduce-scatter (gpsimd blocks on collectives; it has no
            # other work left)
            nc.gpsimd.collective_compute(
                "ReduceScatter", mybir.AluOpType.add,
                replica_groups=[[0, 1], [2, 3], [4, 5], [6, 7]],
                ins=[rs_in1.opt()], outs=[rs_out1.opt()])

            # ======== FFN: W1 (resident weights) + W2 (streamed) ========
            with (
                tc.tile_pool(name="pFF", bufs=1) as pFF,
                tc.tile_pool(name="pO2", bufs=1) as pO2,
                tc.tile_pool(name="pW2q", bufs=2) as pW2q,
                tc.tile_pool(name="psF", bufs=2, space="PSUM") as psF,
                tc.tile_pool(name="psG", bufs=2, space="PSUM") as psG,
            ):
                def ffn_w1(th):
                    FFt = pFF.tile([128, FC, 512], BF16, tag="FF",
                                   name=f"FFt{th}")  # 32KB
                    for fc in range(FC):
                        pf = psF.tile([128, 512], F32, tag="ff")
                        for dc in range(DC):
                            nc.tensor.matmul(
                                pf[:], w1s[:, fc, dc], H1[th][:, dc],
                                start=(dc == 0), stop=(dc == DC - 1))
                        nc.scalar.activation(FFt[:, fc], pf[:], AF.Relu,
                                             bias=b1t[:, fc:fc + 1])
                    return FFt

                def ffn_w2(th, FFt):
                    O2 = pO2.tile([128, DC, 512], F32R, tag="O2",
                                  name=f"O2_{th}")
                    for dout in range(DC):
                        w2t = pW2q.tile([128, FC, 128], BF16, tag="w2")
                        nc.sync.dma_start(w2t[:], w2_d[dout])
                        po2 = psG.tile([128, 512], F32, tag="o2")
                        for fc in range(FC):
                            nc.tensor.matmul(po2[:], w2t[:, fc], FFt[:, fc],
                                             start=(fc == 0),
                                             stop=(fc == FC - 1))
                        # O2[dout] = (po2 + b2[dout]) + h1[dout]
                        nc.vector.scalar_tensor_tensor(
                            O2[:, dout], po2[:], b2t[:, dout:dout + 1],
                            H1[th][:, dout],
                            mybir.AluOpType.add, mybir.AluOpType.add)
                    if dbg is not None:
                        nc.gpsimd.dma_start(
                            dbg["dbg_o2"][:, :, th * 512:(th + 1) * 512],
                            O2[:])
                    def ot_dst(dc):
                        otc = pFF.tile([128, 512], F32, tag="ot", bufs=2,
                                       name="otc")
                        return otc[:]

                    def ot_post(dc, ap):
                        nc.sync.dma_start(
                            outT_d[:, dc, th * 512:(th + 1) * 512], ap)

                    layer_norm(O2[:], ot_dst, g2t, e2t, pLN, psD, stDE,
                               post=ot_post)

                ff0 = ffn_w1(0)
                d_half(1, rs_out1)
                ffn_w2(0, ff0)
                ff1 = ffn_w1(1)
                ffn_w2(1, ff1)


def _pack_inputs(x, Wq, Wk, Wv, Wo, ln1_g, ln1_b, W1, b1, W2, b2, ln2_g, ln2_b):
    """Build the 8 per-core input maps (all host-side numpy)."""
    import ml_dtypes
    bf = ml_dtypes.bfloat16
    f = np.float32
    x = np.asarray(x, f)
    Wq = np.asarray(Wq, f); Wk = np.asarray(Wk, f); Wv = np.asarray(Wv, f)
    Wo = np.asarray(Wo, f); W1 = np.asarray(W1, f); W2 = np.asarray(W2, f)
    in_maps = []
    w1p = np.ascontiguousarray(
        W1.reshape(DC, 128, FC, 128).transpose(1, 2, 0, 3)).astype(bf)
    w2p = np.ascontiguousarray(
        W2.reshape(FC, 128, DC, 128).transpose(2, 1, 0, 3)).astype(bf)
    b1s = np.ascontiguousarray(np.asarray(b1, f).reshape(FC, 128).T)
    b2s = np.ascontiguousarray(np.asarray(b2, f).reshape(DC, 128).T)
    g1s = np.ascontiguousarray(np.asarray(ln1_g, f).reshape(DC, 128).T)
    e1s = np.ascontiguousarray(np.asarray(ln1_b, f).reshape(DC, 128).T)
    g2s = np.ascontiguousarray(np.asarray(ln2_g, f).reshape(DC, 128).T)
    e2s = np.ascontiguousarray(np.asarray(ln2_b, f).reshape(DC, 128).T)
    kk = np.arange(128)[:, None]
    qq = np.arange(128)[None, :]
    maskt = np.ascontiguousarray((kk <= qq).astype(f)).astype(bf)

    for c in range(NCORES):
        b, j = c // 2, c % 2
        hb = j * HL
        xT = np.ascontiguousarray(
            x[b].T.reshape(DC, 128, S).transpose(1, 0, 2)).astype(bf)
        # owned token blocks: {j, j+2} of four 512-blocks
        xm = np.concatenate(
            [x[b, j * 512:(j + 1) * 512],
             x[b, (j + 2) * 512:(j + 3) * 512]]).T  # [D, TLOC]
        xTm = np.ascontiguousarray(
            xm.reshape(DC, 128, TLOC).transpose(1, 0, 2))
        wq = np.stack([np.concatenate([Wq[hb + 2 * p], Wq[hb + 2 * p + 1]], 1)
                       for p in range(NP)])  # [NP, D, 128]
        wq = np.ascontiguousarray(
            wq.reshape(NP, DC, 128, 128).transpose(0, 2, 1, 3)).astype(bf)
        wk = np.stack([np.concatenate([Wk[hb + 2 * p], Wk[hb + 2 * p + 1]], 1)
                       for p in range(NP)])
        wk = np.ascontiguousarray(
            wk.reshape(NP, DC, 128, 128).transpose(0, 2, 1, 3)).astype(bf)
        wv = np.concatenate([Wv[hb + i] for i in range(HL)], 1)  # [D, 512]
        wv = np.ascontiguousarray(
            wv.reshape(DC, 128, 512).transpose(1, 0, 2)).astype(bf)
        wo = Wo[j * 512:(j + 1) * 512]  # [512, D]
        wo = np.ascontiguousarray(
            wo.reshape(NP, 128, DC, 128).transpose(1, 0, 2, 3)).astype(bf)
        in_maps.append({
            "xT": xT, "xTmine": xTm, "wq": wq, "wk": wk, "wv": wv, "wo": wo,
            "w1": w1p, "w2": w2p, "b1s": b1s, "b2s": b2s,
            "g1s": g1s, "e1s": e1s, "g2s": g2s, "e2s": e2s, "maskt": maskt,
        })
    return in_maps


def get_compiled():
    global _COMPILED
    if _COMPILED is None:
        _COMPILED = _build()
    return _COMPILED


def kernel(x, Wq, Wk, Wv, Wo, ln1_g, ln1_b, W1, b1, W2, b2, ln2_g, ln2_b,
           _trace=False):
    nc = get_compiled()
    in_maps = _pack_inputs(x, Wq, Wk, Wv, Wo, ln1_g, ln1_b, W1, b1, W2, b2,
                           ln2_g, ln2_b)
    res = run_bass_kernel_spmd(nc, in_maps, core_ids=list(range(NCORES)),
                               trace=_trace)
    out = np.zeros((B, S, D), np.float32)
    for c in range(NCORES):
        b, j = c // 2, c % 2
        o = res.results[c]["outT"]  # [128, DC, TLOC]
        o = np.asarray(o, np.float32).transpose(1, 0, 2).reshape(D, TLOC)
        out[b, j * 512:(j + 1) * 512, :] = o[:, 0:512].T
        out[b, (j + 2) * 512:(j + 3) * 512, :] = o[:, 512:1024].T
    kernel.last_result = res
    return out


# revision 36
# speedup vs baseline: 1.1473x; 1.1199x over previous
"""Trainium2 Bass kernel for a single transformer decoder layer.

Sharding: 8 cores = 4 batches x 2 head-groups (tensor parallel over heads for
attention; pairwise ReduceScatter; token-split FFN). All activations are kept
feature-major on device. bf16 everywhere on the matmul paths (fp32 PSUM
accumulation, fp32 residual/LN math). FFN W1 is fully SBUF-resident (prefetched
during attention); W2 streams per-dout. Softmax denominator broadcast uses a
tiny PE matmul so the gpsimd queue only runs the collectives.
"""

import sys

for _p in ("/opt/trn_rl_repo",):
    if _p not in sys.path:
        sys.path.insert(0, _p)

import numpy as np

import concourse.bass as bass
import concourse.mybir as mybir
import concourse.tile as tile
from concourse import bacc
from concourse.bass_utils import run_bass_kernel_spmd

# ---- problem constants (hardcoded per spec) ----
B, S, D = 4, 2048, 1024
H, DK, DV, DFF = 16, 64, 64, 4096
EPS = 1e-5
SCALE = 1.0 / 32.0  # 1/sqrt(D)

NCORES = 8
HL = H // 2          # heads per core (local)
NP = HL // 2         # head-pairs per core (4)
TLOC = S // 2        # tokens owned per core after reduce-scatter (1024)
DC = D // 128        # d-model chunks (8)
FC = DFF // 128      # dff chunks (32)
QB = S // 512        # query blocks of 512 (4)

F32 = mybir.dt.float32
F32R = mybir.dt.float32r
BF16 = mybir.dt.bfloat16

DEBUG = False
_COMPILED = None


def _build():
    nc = bacc.Bacc("TRN2", target_bir_lowering=False, debug=False,
                   num_devices=NCORES)

    xT_d = nc.dram_tensor("xT", [128, DC, S], BF16, kind="ExternalInput").ap()
    xTm_d = nc.dram_tensor("xTmine", [128, DC, TLOC], F32,
                           kind="ExternalInput").ap()
    wq_d = nc.dram_tensor("wq", [NP, 128, DC, 128], BF16,
                          kind="ExternalInput").ap()
    wk_d = nc.dram_tensor("wk", [NP, 128, DC, 128], BF16,
                          kind="ExternalInput").ap()
    wv_d = nc.dram_tensor("wv", [128, DC, 512], BF16, kind="ExternalInput").ap()
    wo_d = nc.dram_tensor("wo", [128, NP, DC, 128], BF16,
                          kind="ExternalInput").ap()
    w1_d = nc.dram_tensor("w1", [128, FC, DC, 128], BF16,
                          kind="ExternalInput").ap()
    w2_d = nc.dram_tensor("w2", [DC, 128, FC, 128], BF16,
                          kind="ExternalInput").ap()
    b1_d = nc.dram_tensor("b1s", [128, FC], F32, kind="ExternalInput").ap()
    b2_d = nc.dram_tensor("b2s", [128, DC], F32, kind="ExternalInput").ap()
    g1_d = nc.dram_tensor("g1s", [128, DC], F32, kind="ExternalInput").ap()
    e1_d = nc.dram_tensor("e1s", [128, DC], F32, kind="ExternalInput").ap()
    g2_d = nc.dram_tensor("g2s", [128, DC], F32, kind="ExternalInput").ap()
    e2_d = nc.dram_tensor("e2s", [128, DC], F32, kind="ExternalInput").ap()
    mk_d = nc.dram_tensor("maskt", [128, 128], BF16, kind="ExternalInput").ap()

    outT_d = nc.dram_tensor("outT", [128, DC, TLOC], F32,
                            kind="ExternalOutput").ap()
    dbg = None
    if DEBUG:
        dbg = {
            "dbg_ctx": nc.dram_tensor("dbg_ctx", [128, NP, S], F32,
                                      kind="ExternalOutput").ap(),
            "dbg_h1": nc.dram_tensor("dbg_h1", [128, DC, TLOC], F32,
                                     kind="ExternalOutput").ap(),
            "dbg_o2": nc.dram_tensor("dbg_o2", [128, DC, TLOC], F32,
                                     kind="ExternalOutput").ap(),
        }

    with tile.TileContext(nc) as tc:
        _emit(nc, tc, xT_d, xTm_d, wq_d, wk_d, wv_d, wo_d, w1_d, w2_d,
              b1_d, b2_d, g1_d, e1_d, g2_d, e2_d, mk_d, outT_d, dbg)
    nc.compile()
    return nc


def _emit(nc, tc, xT_d, xTm_d, wq_d, wk_d, wv_d, wo_d, w1_d, w2_d,
          b1_d, b2_d, g1_d, e1_d, g2_d, e2_d, mk_d, outT_d, dbg=None):
    AF = mybir.ActivationFunctionType

    with (
        tc.tile_pool(name="dram", bufs=1, space="DRAM") as dram,
        tc.tile_pool(name="const", bufs=1) as const,
        tc.tile_pool(name="pW1", bufs=1) as pW1,
    ):
        MASKT = const.tile([128, 128], BF16)
        nc.gpsimd.dma_start(MASKT[:], mk_d[:])
        onesf = const.tile([128, 1], F32)
        nc.vector.memset(onesf[:], 1.0)
        ones1 = const.tile([128, 1], F32R)
        nc.vector.tensor_copy(ones1[:], onesf[:])
        onesrf = const.tile([1, 128], F32)
        nc.vector.memset(onesrf[:], 1.0)
        onesr = const.tile([1, 128], F32R)
        nc.vector.tensor_copy(onesr[:], onesrf[:])
        epst = const.tile([1, 1], F32)
        nc.vector.memset(epst[:], EPS)
        g1t = const.tile([128, DC], F32)
        e1t = const.tile([128, DC], F32)
        g2t = const.tile([128, DC], F32)
        e2t = const.tile([128, DC], F32)
        b1t = const.tile([128, FC], F32)
        b2t = const.tile([128, DC], F32)
        for t_, d_ in ((g1t, g1_d), (e1t, e1_d), (g2t, g2_d), (e2t, e2_d),
                       (b1t, b1_d), (b2t, b2_d)):
            nc.sync.dma_start(t_[:], d_[:])

        # resident W1 (prefetched on the scalar engine's DMA queue; it is idle
        # until the first exp so the kicks go out immediately)
        w1s = pW1.tile([128, FC, DC, 128], BF16)
        for q4 in range(4):
            nc.scalar.dma_start(w1s[:, q4 * 8:(q4 + 1) * 8],
                                w1_d[:, q4 * 8:(q4 + 1) * 8])

        F16 = mybir.dt.float16
        rs_in0 = dram.tile([2, D, 512], F16)
        rs_in1 = dram.tile([2, D, 512], F16)
        rs_out0 = dram.tile([D, 512], F16)
        rs_out1 = dram.tile([D, 512], F16)
        dnd = dram.tile([NP, 2, 512], F32)  # softmax denominator bounce

        def layer_norm(src, dst, gt, et, work, psStat, statp, post=None):
            """feature-major LN over features of a [128, DC, 512] block.

            dst may be None; then per-dc outputs stream through a bounce tile
            given by dst_cb(dc, chunk_ap) -> None.
            """
            pmu = psStat.tile([1, 512], F32, tag="stat")
            psq = psStat.tile([1, 512], F32, tag="stat")
            for dc in range(DC):
                sq = work.tile([128, 512], F32R, tag="sq", bufs=2)
                nc.scalar.activation(sq[:], src[:, dc], AF.Square)
                nc.tensor.matmul(pmu[:], ones1[:], src[:, dc],
                                 start=(dc == 0), stop=(dc == DC - 1))
                nc.tensor.matmul(psq[:], ones1[:], sq[:],
                                 start=(dc == 0), stop=(dc == DC - 1))
            mu = statp.tile([1, 512], F32, tag="mu")
            ex2 = statp.tile([1, 512], F32, tag="ex2")
            nc.vector.tensor_scalar_mul(mu[:], pmu[:], 1.0 / D)
            nc.vector.tensor_scalar_mul(ex2[:], psq[:], 1.0 / D)
            var = statp.tile([1, 512], F32, tag="var")
            nc.vector.tensor_mul(var[:], mu[:], mu[:])
            nc.vector.tensor_sub(var[:], ex2[:], var[:])
            srt = statp.tile([1, 512], F32, tag="srt")
            nc.scalar.activation(srt[:], var[:], AF.Sqrt, bias=epst[:])
            rstd = statp.tile([1, 512], F32, tag="rstd")
            nc.vector.reciprocal_approx_fast(out=rstd[:], in_=srt[:])
            nmr = statp.tile([1, 512], F32, tag="nmr")
            nc.vector.tensor_mul(nmr[:], mu[:], rstd[:])
            nc.vector.tensor_scalar_mul(nmr[:], nmr[:], -1.0)
            # broadcast rstd / -mu*rstd across partitions via a PE matmul
            rstf = statp.tile([1, 2, 512], F32R, tag="rbf")
            nc.vector.tensor_copy(rstf[:, 0], rstd[:])
            nc.vector.tensor_copy(rstf[:, 1], nmr[:])
            psb = psStat.tile([128, 2, 512], F32, tag="statb", bufs=1)
            nc.tensor.matmul(psb[:, 0], onesr[:], rstf[:, 0],
                             start=True, stop=True)
            nc.tensor.matmul(psb[:, 1], onesr[:], rstf[:, 1],
                             start=True, stop=True)
            for dc in range(DC):
                xh = work.tile([128, 512], F32, tag="xh", bufs=2)
                nc.vector.tensor_mul(xh[:], src[:, dc], psb[:, 0])
                nc.vector.tensor_add(xh[:], xh[:], psb[:, 1])
                ap = dst(dc)
                nc.scalar.activation(ap, xh[:], AF.Identity,
                                     bias=et[:, dc:dc + 1],
                                     scale=gt[:, dc:dc + 1])
                if post is not None:
                    post(dc, ap)

        # ============ attention: projections + qb-major attention ============
        with (
            tc.tile_pool(name="pQKT", bufs=1) as pQKT,
            tc.tile_pool(name="pV", bufs=1) as pV,
        ):
            QT = pQKT.tile([128, NP, S], BF16, tag="QT")          # 16KB
            KT = pQKT.tile([128, NP, S], BF16, tag="KT")          # 16KB
            V = pV.tile([128, S // 128, HL * 65], BF16, tag="V")  # 16.3KB
            nc.vector.tensor_copy(
                V[:].rearrange("p t (h c) -> p t h c", c=65)[:, :, :, 64:65],
                onesf[:, None, None, :].to_broadcast((128, S // 128, HL, 1)))

            with (
                tc.tile_pool(name="pX", bufs=1) as pX,
                tc.tile_pool(name="pWQK", bufs=2) as pWQK,
            ):
                X = pX.tile([128, DC, S], BF16, tag="X")          # 32KB
                for dc in range(DC):
                    nc.sync.dma_start(X[:, dc], xT_d[:, dc])

                # Q/K projections, dc-outer so PE starts on the first X chunk
                with tc.tile_pool(name="psP", bufs=8, space="PSUM") as psP:
                    for p in range(NP):
                        wqt = pWQK.tile([128, DC, 128], BF16, tag="wq")
                        wkt = pWQK.tile([128, DC, 128], BF16, tag="wk")
                        nc.gpsimd.dma_start(wqt[:], wq_d[p])
                        nc.gpsimd.dma_start(wkt[:], wk_d[p])
                        pqs = [psP.tile([128, 512], F32, tag="proj",
                                        name=f"pq_{i}") for i in range(8)]
                        for dc in range(DC):
                            for tb in range(QB):
                                nc.tensor.matmul(
                                    pqs[tb][:], wqt[:, dc],
                                    X[:, dc, tb * 512:(tb + 1) * 512],
                                    start=(dc == 0), stop=(dc == DC - 1))
                            for tb in range(QB):
                                nc.tensor.matmul(
                                    pqs[4 + tb][:], wkt[:, dc],
                                    X[:, dc, tb * 512:(tb + 1) * 512],
                                    start=(dc == 0), stop=(dc == DC - 1))
                        for tb in range(QB):
                            tsl = slice(tb * 512, (tb + 1) * 512)
                            nc.vector.tensor_copy(QT[:, p, tsl], pqs[tb][:])
                            nc.vector.tensor_copy(KT[:, p, tsl], pqs[4 + tb][:])

                # V projection (needs all of X)
                with (
                    tc.tile_pool(name="psV", bufs=3, space="PSUM") as psV,
                    tc.tile_pool(name="pWV", bufs=1) as pWV,
                ):
                    wvt = pWV.tile([128, DC, 512], BF16, tag="wv")
                    nc.gpsimd.dma_start(wvt[:], wv_d[:])
                    for tt in range(S // 128):
                        pv = psV.tile([128, 512], F32, tag="pv")
                        for dc in range(DC):
                            nc.tensor.matmul(pv[:],
                                             X[:, dc, tt * 128:(tt + 1) * 128],
                                             wvt[:, dc],
                                             start=(dc == 0), stop=(dc == DC - 1))
                        nc.vector.tensor_copy(
                            V[:, tt].rearrange("p (h c) -> p h c",
                                               c=65)[:, :, 0:64],
                            pv[:].rearrange("p (h c) -> p h c", c=64))

            # ---- attention, qb-outer; Wo + reduce-scatter interleaved ----
            with (
                tc.tile_pool(name="pCTX", bufs=1) as pCTX,
                tc.tile_pool(name="pWO", bufs=1) as pWO,
                tc.tile_pool(name="pE", bufs=3) as pE,
                tc.tile_pool(name="pAO", bufs=3) as pAO,
                tc.tile_pool(name="stB", bufs=2) as stB,
                tc.tile_pool(name="psS", bufs=2, space="PSUM") as psS,
                tc.tile_pool(name="psC", bufs=4, space="PSUM") as psC,
            ):
                CTX = pCTX.tile([128, NP, S], BF16, tag="CTX")    # 16KB
                wot = pWO.tile([128, NP, DC, 128], BF16, tag="wo")
                nc.gpsimd.dma_start(wot[:], wo_d[:])

                def normalize(cts, p, qsl):
                    """softmax-normalize one p's two head columns into CTX.

                    The [1,512] denominator is broadcast to 64 partitions via
                    a DRAM round-trip DMA: no PSUM bank, no compute engine.
                    """
                    for i, (row0, (cxt, den)) in enumerate(
                            ((0, cts[0]), (64, cts[1]))):
                        denB = stB.tile([64, 512], F32, tag="denB", bufs=2)
                        nc.sync.dma_start(dnd[p, i][None, :], den[:])
                        nc.sync.dma_start(
                            denB[:],
                            dnd[p, i][None, :].to_broadcast((64, 512)))
                        recb = stB.tile([64, 512], F32, tag="recb")
                        nc.vector.reciprocal_approx_fast(out=recb[:],
                                                         in_=denB[:])
                        nc.vector.tensor_mul(CTX[row0:row0 + 64, p, qsl],
                                             cxt[0:64, :], recb[:])

                for qb in range(QB):
                    qsl = slice(qb * 512, (qb + 1) * 512)
                    nkc = 4 * (qb + 1)
                    pending = []
                    for p in range(NP):
                        ctxA = psC.tile([65, 512], F32, tag="ctx")
                        ctxB = psC.tile([65, 512], F32, tag="ctx")

                        def emit_ctx(kc, eAB, off):
                            st, sp = (kc == 0), (kc == nkc - 1)
                            nc.tensor.matmul(
                                ctxA[:, off:],
                                V[:, kc, 2 * p * 65:(2 * p + 1) * 65],
                                eAB[:, 0, off:], start=st, stop=sp)
                            nc.tensor.matmul(
                                ctxB[:, off:],
                                V[:, kc, (2 * p + 1) * 65:(2 * p + 2) * 65],
                                eAB[:, 1, off:], start=st, stop=sp)

                        # ctx matmuls lag the scores by one kc so the PE
                        # stream never waits on exp/mask of the same kc
                        prev = None
                        first = True
                        for kc in range(nkc):
                            ksl = slice(kc * 128, (kc + 1) * 128)
                            diag = kc >= 4 * qb
                            off = (kc - 4 * qb) * 128 if diag else 0
                            qtr = slice(qb * 512 + off, (qb + 1) * 512)
                            sAB = psS.tile([128, 2, 512], F32, tag="sc")
                            nc.tensor.matmul(sAB[:, 0, off:],
                                             KT[0:64, p, ksl],
                                             QT[0:64, p, qtr],
                                             start=True, stop=True)
                            nc.tensor.matmul(sAB[:, 1, off:],
                                             KT[64:128, p, ksl],
                                             QT[64:128, p, qtr],
                                             start=True, stop=True)
                            if prev is not None:
                                emit_ctx(*prev)
                            elif first and pending:
                                # normalize of p-1 emitted early inside p's
                                # loop so its DVE chain overlaps p's compute
                                first = False
                                pn = pending.pop(0)
                                normalize(pn[0], pn[1], qsl)
                            eAB = pE.tile([128, 2, 512], BF16, tag="E")
                            nc.scalar.activation(eAB[:, :, off:],
                                                 sAB[:, :, off:],
                                                 AF.Exp, scale=SCALE)
                            if diag:
                                nc.vector.tensor_mul(
                                    eAB[:, :, off:off + 128],
                                    eAB[:, :, off:off + 128],
                                    MASKT[:, None, :].to_broadcast(
                                        (128, 2, 128)))
                            prev = (kc, eAB, off)
                        emit_ctx(*prev)
                        dens = []
                        for i, cxt in enumerate((ctxA, ctxB)):
                            den = stB.tile([1, 512], F32, tag="den",
                                           name=f"den{i}", bufs=4)
                            nc.vector.tensor_copy(den[:], cxt[64:65, :])
                            dens.append((cxt, den))
                        pending.append((dens, p))
                    for pn in pending:
                        normalize(pn[0], pn[1], qsl)
                    pending = []
                    # Wo partial for this token block
                    rsdst = rs_in0 if qb < 2 else rs_in1
                    for dout in range(DC):
                        po = psS.tile([128, 512], F32, tag="sc", name="po")
                        for p in range(NP):
                            nc.tensor.matmul(po[:], wot[:, p, dout],
                                             CTX[:, p, qsl],
                                             start=(p == 0), stop=(p == NP - 1))
                        ao = pAO.tile([128, 512], mybir.dt.float16, tag="ao")
                        nc.vector.tensor_copy(ao[:], po[:])
                        nc.sync.dma_start(
                            rsdst[qb % 2, dout * 128:(dout + 1) * 128, :],
                            ao[:])
                    if qb == 1:
                        nc.gpsimd.collective_compute(
                            "ReduceScatter", mybir.AluOpType.add,
                            replica_groups=[[0, 1], [2, 3], [4, 5], [6, 7]],
                            ins=[rs_in0.opt()], outs=[rs_out0.opt()])
                if dbg is not None:
                    nc.gpsimd.dma_start(dbg["dbg_ctx"][:], CTX[:])

        # ======== LN1 halves + FFN ========
        with (
            tc.tile_pool(name="pH1", bufs=1) as pH1,
            tc.tile_pool(name="pAOr", bufs=1) as pAOr,
            tc.tile_pool(name="pLN", bufs=1) as pLN,
            tc.tile_pool(name="stDE", bufs=1) as stDE,
            tc.tile_pool(name="psD", bufs=2, space="PSUM") as psD,
        ):
            H1 = [pH1.tile([128, DC, 512], BF16, tag=f"H1_{h}",
                           name=f"H1_{h}") for h in range(2)]

            def d_half(h, rso):
                aor = pAOr.tile([128, DC, 512], F32R, tag="AOr",
                                name=f"AOr{h}")
                nc.gpsimd.dma_start(
                    aor[:], rso.rearrange("(dc p) t -> p dc t", p=128))
                xm = pAOr.tile([128, DC, 512], F32, tag="XM",
                               name=f"XMt{h}")
                nc.sync.dma_start(xm[:], xTm_d[:, :, h * 512:(h + 1) * 512])
                nc.vector.tensor_add(aor[:], aor[:], xm[:])
                layer_norm(aor[:], lambda dc: H1[h][:, dc], g1t, e1t,
                           pLN, psD, stDE)
                if dbg is not None:
                    nc.gpsimd.dma_start(
                        dbg["dbg_h1"][:, :, h * 512:(h + 1) * 512],
                        H1[h][:])

            d_half(0, rs_out0)
            # second reduce-scatter (gpsimd blocks on collectives; it has no
            # other work left)
            nc.gpsimd.collective_compute(
                "ReduceScatter", mybir.AluOpType.add,
                replica_groups=[[0, 1], [2, 3], [4, 5], [6, 7]],
                ins=[rs_in1.opt()], outs=[rs_out1.opt()])

            # ======== FFN: W1 (resident weights) + W2 (streamed) ========
            with (
                tc.tile_pool(name="pFF", bufs=1) as pFF,
                tc.tile_pool(name="pO2", bufs=1) as pO2,
                tc.tile_pool(name="pW2q", bufs=2) as pW2q,
                tc.tile_pool(name="psF", bufs=2, space="PSUM") as psF,
                tc.tile_pool(name="psG", bufs=2, space="PSUM") as psG,
            ):
                def ffn_w1(th):
                    FFt = pFF.tile([128, FC, 512], BF16, tag="FF",
                                   name=f"FFt{th}")  # 32KB
                    for fc in range(FC):
                        pf = psF.tile([128, 512], F32, tag="ff")
                        for dc in range(DC):
                            nc.tensor.matmul(
                                pf[:], w1s[:, fc, dc], H1[th][:, dc],
                                start=(dc == 0), stop=(dc == DC - 1))
                        nc.scalar.activation(FFt[:, fc], pf[:], AF.Relu,
                                             bias=b1t[:, fc:fc + 1])
                    return FFt

                def ffn_w2(th, FFt):
                    O2 = pO2.tile([128, DC, 512], F32R, tag="O2",
                                  name=f"O2_{th}")
                    for dout in range(DC):
                        w2t = pW2q.tile([128, FC, 128], BF16, tag="w2")
                        nc.sync.dma_start(w2t[:], w2_d[dout])
                        po2 = psG.tile([128, 512], F32, tag="o2")
                        for fc in range(FC):
                            nc.tensor.matmul(po2[:], w2t[:, fc], FFt[:, fc],
                                             start=(fc == 0),
                                             stop=(fc == FC - 1))
                        # O2[dout] = (po2 + b2[dout]) + h1[dout]
                        nc.vector.scalar_tensor_tensor(
                            O2[:, dout], po2[:], b2t[:, dout:dout + 1],
                            H1[th][:, dout],
                            mybir.AluOpType.add, mybir.AluOpType.add)
                    if dbg is not None:
                        nc.gpsimd.dma_start(
                            dbg["dbg_o2"][:, :, th * 512:(th + 1) * 512],
                            O2[:])
                    def ot_dst(dc):
                        otc = pFF.tile([128, 512], F32, tag="ot", bufs=2,
                                       name="otc")
                        return otc[:]

                    def ot_post(dc, ap):
                        nc.sync.dma_start(
                            outT_d[:, dc, th * 512:(th + 1) * 512], ap)

                    layer_norm(O2[:], ot_dst, g2t, e2t, pLN, psD, stDE,
                               post=ot_post)

                ff0 = ffn_w1(0)
                d_half(1, rs_out1)
                ffn_w2(0, ff0)
                ff1 = ffn_w1(1)
                ffn_w2(1, ff1)


def _pack_inputs(x, Wq, Wk, Wv, Wo, ln1_g, ln1_b, W1, b1, W2, b2, ln2_g, ln2_b):
    """Build the 8 per-core input maps (all host-side numpy)."""
    import ml_dtypes
    bf = ml_dtypes.bfloat16
    f = np.float32
    x = np.asarray(x, f)
    Wq = np.asarray(Wq, f); Wk = np.asarray(Wk, f); Wv = np.asarray(Wv, f)
    Wo = np.asarray(Wo, f); W1 = np.asarray(W1, f); W2 = np.asarray(W2, f)
    in_maps = []
    w1p = np.ascontiguousarray(
        W1.reshape(DC, 128, FC, 128).transpose(1, 2, 0, 3)).astype(bf)
    w2p = np.ascontiguousarray(
        W2.reshape(FC, 128, DC, 128).transpose(2, 1, 0, 3)).astype(bf)
    b1s = np.ascontiguousarray(np.asarray(b1, f).reshape(FC, 128).T)
    b2s = np.ascontiguousarray(np.asarray(b2, f).reshape(DC, 128).T)
    g1s = np.ascontiguousarray(np.asarray(ln1_g, f).reshape(DC, 128).T)
    e1s = np.ascontiguousarray(np.asarray(ln1_b, f).reshape(DC, 128).T)
    g2s = np.ascontiguousarray(np.asarray(ln2_g, f).reshape(DC, 128).T)
    e2s = np.ascontiguousarray(np.asarray(ln2_b, f).reshape(DC, 128).T)
    kk = np.arange(128)[:, None]
    qq = np.arange(128)[None, :]
    maskt = np.ascontiguousarray((kk <= qq).astype(f)).astype(bf)

    for c in range(NCORES):
        b, j = c // 2, c % 2
        hb = j * HL
        xT = np.ascontiguousarray(
            x[b].T.reshape(DC, 128, S).transpose(1, 0, 2)).astype(bf)
        # owned token blocks: {j, j+2} of four 512-blocks
        xm = np.concatenate(
            [x[b, j * 512:(j + 1) * 512],
             x[b, (j + 2) * 512:(j + 3) * 512]]).T  # [D, TLOC]
        xTm = np.ascontiguousarray(
            xm.reshape(DC, 128, TLOC).transpose(1, 0, 2))
        wq = np.stack([np.concatenate([Wq[hb + 2 * p], Wq[hb + 2 * p + 1]], 1)
                       for p in range(NP)])  # [NP, D, 128]
        wq = np.ascontiguousarray(
            wq.reshape(NP, DC, 128, 128).transpose(0, 2, 1, 3)).astype(bf)
        wk = np.stack([np.concatenate([Wk[hb + 2 * p], Wk[hb + 2 * p + 1]], 1)
                       for p in range(NP)])
        wk = np.ascontiguousarray(
            wk.reshape(NP, DC, 128, 128).transpose(0, 2, 1, 3)).astype(bf)
        wv = np.concatenate([Wv[hb + i] for i in range(HL)], 1)  # [D, 512]
        wv = np.ascontiguousarray(
            wv.reshape(DC, 128, 512).transpose(1, 0, 2)).astype(bf)
        wo = Wo[j * 512:(j + 1) * 512]  # [512, D]
        wo = np.ascontiguousarray(
            wo.reshape(NP, 128, DC, 128).transpose(1, 0, 2, 3)).astype(bf)
        in_maps.append({
            "xT": xT, "xTmine": xTm, "wq": wq, "wk": wk, "wv": wv, "wo": wo,
            "w1": w1p, "w2": w2p, "b1s": b1s, "b2s": b2s,
            "g1s": g1s, "e1s": e1s, "g2s": g2s, "e2s": e2s, "maskt": maskt,
        })
    return in_maps


def get_compiled():
    global _COMPILED
    if _COMPILED is None:
        _COMPILED = _build()
    return _COMPILED


def kernel(x, Wq, Wk, Wv, Wo, ln1_g, ln1_b, W1, b1, W2, b2, ln2_g, ln2_b,
           _trace=False):
    nc = get_compiled()
    in_maps = _pack_inputs(x, Wq, Wk, Wv, Wo, ln1_g, ln1_b, W1, b1, W2, b2,
                           ln2_g, ln2_b)
    res = run_bass_kernel_spmd(nc, in_maps, core_ids=list(range(NCORES)),
                               trace=_trace)
    out = np.zeros((B, S, D), np.float32)
    for c in range(NCORES):
        b, j = c // 2, c % 2
        o = res.results[c]["outT"]  # [128, DC, TLOC]
        o = np.asarray(o, np.float32).transpose(1, 0, 2).reshape(D, TLOC)
        out[b, j * 512:(j + 1) * 512, :] = o[:, 0:512].T
        out[b, (j + 2) * 512:(j + 3) * 512, :] = o[:, 512:1024].T
    kernel.last_result = res
    return out


# revision 42
# speedup vs baseline: 1.1806x; 1.0291x over previous
"""Trainium2 Bass kernel for a single transformer decoder layer.

Sharding: 8 cores = 4 batches x 2 head-groups (tensor parallel over heads for
attention; pairwise ReduceScatter; token-split FFN). All activations are kept
feature-major on device. bf16 everywhere on the matmul paths (fp32 PSUM
accumulation, fp32 residual/LN math). FFN W1 is fully SBUF-resident (prefetched
during attention); W2 streams per-dout. Softmax denominator broadcast uses a
tiny PE matmul so the gpsimd queue only runs the collectives.
"""

import sys

for _p in ("/opt/trn_rl_repo",):
    if _p not in sys.path:
        sys.path.insert(0, _p)

import numpy as np

import concourse.bass as bass
import concourse.mybir as mybir
import concourse.tile as tile
from concourse import bacc
from concourse.bass_utils import run_bass_kernel_spmd

# ---- problem constants (hardcoded per spec) ----
B, S, D = 4, 2048, 1024
H, DK, DV, DFF = 16, 64, 64, 4096
EPS = 1e-5
SCALE = 1.0 / 32.0  # 1/sqrt(D)

NCORES = 8
HL = H // 2          # heads per core (local)
NP = HL // 2         # head-pairs per core (4)
TLOC = S // 2        # tokens owned per core after reduce-scatter (1024)
DC = D // 128        # d-model chunks (8)
FC = DFF // 128      # dff chunks (32)
QB = S // 512        # query blocks of 512 (4)

F32 = mybir.dt.float32
F32R = mybir.dt.float32r
BF16 = mybir.dt.bfloat16

DEBUG = False
_COMPILED = None


def _build():
    nc = bacc.Bacc("TRN2", target_bir_lowering=False, debug=False,
                   num_devices=NCORES)

    xT_d = nc.dram_tensor("xT", [128, DC, S], BF16, kind="ExternalInput").ap()
    xTm_d = nc.dram_tensor("xTmine", [128, DC, TLOC], F32R,
                           kind="ExternalInput").ap()
    wq_d = nc.dram_tensor("wq", [NP, 128, DC, 128], BF16,
                          kind="ExternalInput").ap()
    wk_d = nc.dram_tensor("wk", [NP, 128, DC, 128], BF16,
                          kind="ExternalInput").ap()
    wv_d = nc.dram_tensor("wv", [128, DC, 512], BF16, kind="ExternalInput").ap()
    wo_d = nc.dram_tensor("wo", [128, NP, DC, 128], BF16,
                          kind="ExternalInput").ap()
    w1_d = nc.dram_tensor("w1", [128, FC, DC, 128], BF16,
                          kind="ExternalInput").ap()
    w2_d = nc.dram_tensor("w2", [DC, 128, FC, 128], BF16,
                          kind="ExternalInput").ap()
    b1_d = nc.dram_tensor("b1s", [128, FC], F32, kind="ExternalInput").ap()
    b2_d = nc.dram_tensor("b2s", [128, DC], F32, kind="ExternalInput").ap()
    g1_d = nc.dram_tensor("g1s", [128, DC], F32, kind="ExternalInput").ap()
    e1_d = nc.dram_tensor("e1s", [128, DC], F32, kind="ExternalInput").ap()
    g2_d = nc.dram_tensor("g2s", [128, DC], F32, kind="ExternalInput").ap()
    e2_d = nc.dram_tensor("e2s", [128, DC], F32, kind="ExternalInput").ap()
    mk_d = nc.dram_tensor("maskt", [128, 128], BF16, kind="ExternalInput").ap()

    outT_d = nc.dram_tensor("outT", [128, DC, TLOC], F32,
                            kind="ExternalOutput").ap()
    dbg = None
    if DEBUG:
        dbg = {
            "dbg_ctx": nc.dram_tensor("dbg_ctx", [128, NP, S], F32,
                                      kind="ExternalOutput").ap(),
            "dbg_h1": nc.dram_tensor("dbg_h1", [128, DC, TLOC], F32,
                                     kind="ExternalOutput").ap(),
            "dbg_o2": nc.dram_tensor("dbg_o2", [128, DC, TLOC], F32,
                                     kind="ExternalOutput").ap(),
        }

    with tile.TileContext(nc) as tc:
        _emit(nc, tc, xT_d, xTm_d, wq_d, wk_d, wv_d, wo_d, w1_d, w2_d,
              b1_d, b2_d, g1_d, e1_d, g2_d, e2_d, mk_d, outT_d, dbg)
    nc.compile()
    return nc


def _emit(nc, tc, xT_d, xTm_d, wq_d, wk_d, wv_d, wo_d, w1_d, w2_d,
          b1_d, b2_d, g1_d, e1_d, g2_d, e2_d, mk_d, outT_d, dbg=None):
    AF = mybir.ActivationFunctionType

    with (
        tc.tile_pool(name="dram", bufs=1, space="DRAM") as dram,
        tc.tile_pool(name="const", bufs=1) as const,
        tc.tile_pool(name="pW1", bufs=1) as pW1,
    ):
        MASKT = const.tile([128, 128], BF16)
        nc.gpsimd.dma_start(MASKT[:], mk_d[:])
        onesf = const.tile([128, 1], F32)
        nc.vector.memset(onesf[:], 1.0)
        ones1 = const.tile([128, 1], F32R)
        nc.vector.tensor_copy(ones1[:], onesf[:])
        onesrf = const.tile([1, 128], F32)
        nc.vector.memset(onesrf[:], 1.0)
        onesr = const.tile([1, 128], F32R)
        nc.vector.tensor_copy(onesr[:], onesrf[:])
        epst = const.tile([1, 1], F32)
        nc.vector.memset(epst[:], EPS)
        g1t = const.tile([128, DC], F32)
        e1t = const.tile([128, DC], F32)
        g2t = const.tile([128, DC], F32)
        e2t = const.tile([128, DC], F32)
        b1t = const.tile([128, FC], F32)
        b2t = const.tile([128, DC], F32)
        for t_, d_ in ((g1t, g1_d), (e1t, e1_d), (g2t, g2_d), (e2t, e2_d),
                       (b1t, b1_d), (b2t, b2_d)):
            nc.sync.dma_start(t_[:], d_[:])

        # resident W1 (prefetched on the scalar engine's DMA queue; it is idle
        # until the first exp so the kicks go out immediately)
        w1s = pW1.tile([128, FC, DC, 128], BF16)
        for q4 in range(4):
            nc.scalar.dma_start(w1s[:, q4 * 8:(q4 + 1) * 8],
                                w1_d[:, q4 * 8:(q4 + 1) * 8])

        F16 = mybir.dt.float16
        rs_in0 = dram.tile([2, D, 512], F16)
        rs_in1 = dram.tile([2, D, 512], F16)
        rs_out0 = dram.tile([D, 512], F16)
        rs_out1 = dram.tile([D, 512], F16)
        dnd = dram.tile([NP, 2, 512], F32)  # softmax denominator bounce

        def layer_norm(src, dst, gt, et, work, psStat, statp, post=None):
            """feature-major LN over features of a [128, DC, 512] block.

            dst may be None; then per-dc outputs stream through a bounce tile
            given by dst_cb(dc, chunk_ap) -> None.
            """
            pmu = psStat.tile([1, 512], F32, tag="stat")
            psq = psStat.tile([1, 512], F32, tag="stat")
            for dc in range(DC):
                sq = work.tile([128, 512], F32R, tag="sq", bufs=2)
                nc.scalar.activation(sq[:], src[:, dc], AF.Square)
                nc.tensor.matmul(pmu[:], ones1[:], src[:, dc],
                                 start=(dc == 0), stop=(dc == DC - 1))
                nc.tensor.matmul(psq[:], ones1[:], sq[:],
                                 start=(dc == 0), stop=(dc == DC - 1))
            mu = statp.tile([1, 512], F32, tag="mu")
            ex2 = statp.tile([1, 512], F32, tag="ex2")
            nc.vector.tensor_scalar_mul(mu[:], pmu[:], 1.0 / D)
            nc.vector.tensor_scalar_mul(ex2[:], psq[:], 1.0 / D)
            var = statp.tile([1, 512], F32, tag="var")
            nc.vector.tensor_mul(var[:], mu[:], mu[:])
            nc.vector.tensor_sub(var[:], ex2[:], var[:])
            srt = statp.tile([1, 512], F32, tag="srt")
            nc.scalar.activation(srt[:], var[:], AF.Sqrt, bias=epst[:])
            rstd = statp.tile([1, 512], F32, tag="rstd")
            nc.vector.reciprocal_approx_fast(out=rstd[:], in_=srt[:])
            nmr = statp.tile([1, 512], F32, tag="nmr")
            nc.vector.tensor_mul(nmr[:], mu[:], rstd[:])
            nc.vector.tensor_scalar_mul(nmr[:], nmr[:], -1.0)
            # broadcast rstd / -mu*rstd across partitions via a PE matmul
            rstf = statp.tile([1, 2, 512], F32R, tag="rbf")
            nc.vector.tensor_copy(rstf[:, 0], rstd[:])
            nc.vector.tensor_copy(rstf[:, 1], nmr[:])
            psb = psStat.tile([128, 2, 512], F32, tag="statb", bufs=1)
            nc.tensor.matmul(psb[:, 0], onesr[:], rstf[:, 0],
                             start=True, stop=True)
            nc.tensor.matmul(psb[:, 1], onesr[:], rstf[:, 1],
                             start=True, stop=True)
            for dc in range(DC):
                xh = work.tile([128, 512], F32, tag="xh", bufs=2)
                nc.vector.tensor_mul(xh[:], src[:, dc], psb[:, 0])
                nc.vector.tensor_add(xh[:], xh[:], psb[:, 1])
                ap = dst(dc)
                nc.scalar.activation(ap, xh[:], AF.Identity,
                                     bias=et[:, dc:dc + 1],
                                     scale=gt[:, dc:dc + 1])
                if post is not None:
                    post(dc, ap)

        # ============ attention: projections + qb-major attention ============
        with (
            tc.tile_pool(name="pQKT", bufs=1) as pQKT,
            tc.tile_pool(name="pV", bufs=1) as pV,
        ):
            QT = pQKT.tile([128, NP, S], BF16, tag="QT")          # 16KB
            KT = pQKT.tile([128, NP, S], BF16, tag="KT")          # 16KB
            V = pV.tile([128, S // 128, HL * 65], BF16, tag="V")  # 16.3KB
            nc.vector.tensor_copy(
                V[:].rearrange("p t (h c) -> p t h c", c=65)[:, :, :, 64:65],
                onesf[:, None, None, :].to_broadcast((128, S // 128, HL, 1)))

            with (
                tc.tile_pool(name="pX", bufs=1) as pX,
                tc.tile_pool(name="pWQK", bufs=2) as pWQK,
            ):
                X = pX.tile([128, DC, S], BF16, tag="X")          # 32KB
                for dc in range(DC):
                    nc.sync.dma_start(X[:, dc], xT_d[:, dc])

                # Q/K projections, dc-outer so PE starts on the first X chunk
                with tc.tile_pool(name="psP", bufs=8, space="PSUM") as psP:
                    for p in range(NP):
                        wqt = pWQK.tile([128, DC, 128], BF16, tag="wq")
                        wkt = pWQK.tile([128, DC, 128], BF16, tag="wk")
                        nc.gpsimd.dma_start(wqt[:], wq_d[p])
                        nc.gpsimd.dma_start(wkt[:], wk_d[p])
                        pqs = [psP.tile([128, 512], F32, tag="proj",
                                        name=f"pq_{i}") for i in range(8)]
                        for dc in range(DC):
                            for tb in range(QB):
                                nc.tensor.matmul(
                                    pqs[tb][:], wqt[:, dc],
                                    X[:, dc, tb * 512:(tb + 1) * 512],
                                    start=(dc == 0), stop=(dc == DC - 1))
                            for tb in range(QB):
                                nc.tensor.matmul(
                                    pqs[4 + tb][:], wkt[:, dc],
                                    X[:, dc, tb * 512:(tb + 1) * 512],
                                    start=(dc == 0), stop=(dc == DC - 1))
                        for tb in range(QB):
                            tsl = slice(tb * 512, (tb + 1) * 512)
                            nc.vector.tensor_copy(QT[:, p, tsl], pqs[tb][:])
                            nc.vector.tensor_copy(KT[:, p, tsl], pqs[4 + tb][:])

                # V projection (needs all of X)
                with (
                    tc.tile_pool(name="psV", bufs=3, space="PSUM") as psV,
                    tc.tile_pool(name="pWV", bufs=1) as pWV,
                ):
                    wvt = pWV.tile([128, DC, 512], BF16, tag="wv")
                    nc.gpsimd.dma_start(wvt[:], wv_d[:])
                    for tt in range(S // 128):
                        pv = psV.tile([128, 512], F32, tag="pv")
                        for dc in range(DC):
                            nc.tensor.matmul(pv[:],
                                             X[:, dc, tt * 128:(tt + 1) * 128],
                                             wvt[:, dc],
                                             start=(dc == 0), stop=(dc == DC - 1))
                        nc.vector.tensor_copy(
                            V[:, tt].rearrange("p (h c) -> p h c",
                                               c=65)[:, :, 0:64],
                            pv[:].rearrange("p (h c) -> p h c", c=64))

            # ---- attention, qb-outer; Wo + reduce-scatter interleaved ----
            with (
                tc.tile_pool(name="pCTX", bufs=1) as pCTX,
                tc.tile_pool(name="pWO", bufs=1) as pWO,
                tc.tile_pool(name="pE", bufs=3) as pE,
                tc.tile_pool(name="pAO", bufs=3) as pAO,
                tc.tile_pool(name="stB", bufs=2) as stB,
                tc.tile_pool(name="psS", bufs=2, space="PSUM") as psS,
                tc.tile_pool(name="psC", bufs=4, space="PSUM") as psC,
            ):
                CTX = pCTX.tile([128, NP, S], BF16, tag="CTX")    # 16KB
                wot = pWO.tile([128, NP, DC, 128], BF16, tag="wo")
                nc.gpsimd.dma_start(wot[:], wo_d[:])

                def normalize(cts, p, qsl, use_pe=False):
                    """softmax-normalize one p's two head columns into CTX.

                    The [1,512] denominator broadcast to 64 partitions goes
                    via a DRAM round-trip DMA (no PSUM bank, no engine); when
                    latency matters (use_pe, for the last p before Wo) a PE
                    matmul against a ones column does it instead.
                    """
                    for i, (row0, (cxt, den)) in enumerate(
                            ((0, cts[0]), (64, cts[1]))):
                        if use_pe:
                            denr = stB.tile([1, 512], F32R, tag="denr",
                                            bufs=2)
                            nc.vector.tensor_copy(denr[:], cxt[64:65, :])
                            denb = psS.tile([128, 512], F32, tag="sc",
                                            name="denb")
                            nc.tensor.matmul(denb[0:64, :], onesr[:, 0:64],
                                             denr[:], start=True, stop=True)
                            recb = stB.tile([64, 512], F32, tag="recb")
                            nc.vector.reciprocal_approx_fast(
                                out=recb[:], in_=denb[0:64, :])
                        else:
                            denB = stB.tile([64, 512], F32, tag="denB",
                                            bufs=2)
                            nc.sync.dma_start(dnd[p, i][None, :], den[:])
                            nc.sync.dma_start(
                                denB[:],
                                dnd[p, i][None, :].to_broadcast((64, 512)))
                            recb = stB.tile([64, 512], F32, tag="recb")
                            nc.vector.reciprocal_approx_fast(out=recb[:],
                                                             in_=denB[:])
                        nc.vector.tensor_mul(CTX[row0:row0 + 64, p, qsl],
                                             cxt[0:64, :], recb[:])

                def wo_block(qb):
                    qsl = slice(qb * 512, (qb + 1) * 512)
                    rsdst = rs_in0 if qb < 2 else rs_in1
                    for dout in range(DC):
                        po = psS.tile([128, 512], F32, tag="sc", name="po")
                        for p in range(NP):
                            nc.tensor.matmul(po[:], wot[:, p, dout],
                                             CTX[:, p, qsl],
                                             start=(p == 0), stop=(p == NP - 1))
                        ao = pAO.tile([128, 512], mybir.dt.float16, tag="ao")
                        nc.vector.tensor_copy(ao[:], po[:])
                        nc.sync.dma_start(
                            rsdst[qb % 2, dout * 128:(dout + 1) * 128, :],
                            ao[:])
                    if qb == 1:
                        nc.gpsimd.collective_compute(
                            "ReduceScatter", mybir.AluOpType.add,
                            replica_groups=[[0, 1], [2, 3], [4, 5], [6, 7]],
                            ins=[rs_in0.opt()], outs=[rs_out0.opt()])

                pending = []
                deferred_wo = [None]
                for qb in range(QB):
                    qsl = slice(qb * 512, (qb + 1) * 512)
                    nkc = 4 * (qb + 1)
                    for p in range(NP):
                        ctxA = psC.tile([65, 512], F32, tag="ctx")
                        ctxB = psC.tile([65, 512], F32, tag="ctx")

                        def emit_ctx(kc, eAB, off):
                            st, sp = (kc == 0), (kc == nkc - 1)
                            nc.tensor.matmul(
                                ctxA[:, off:],
                                V[:, kc, 2 * p * 65:(2 * p + 1) * 65],
                                eAB[:, 0, off:], start=st, stop=sp)
                            nc.tensor.matmul(
                                ctxB[:, off:],
                                V[:, kc, (2 * p + 1) * 65:(2 * p + 2) * 65],
                                eAB[:, 1, off:], start=st, stop=sp)

                        # ctx matmuls lag the scores by one kc so the PE
                        # stream never waits on exp/mask of the same kc
                        prev = None
                        first = True
                        for kc in range(nkc):
                            ksl = slice(kc * 128, (kc + 1) * 128)
                            diag = kc >= 4 * qb
                            off = (kc - 4 * qb) * 128 if diag else 0
                            qtr = slice(qb * 512 + off, (qb + 1) * 512)
                            sAB = psS.tile([128, 2, 512], F32, tag="sc")
                            nc.tensor.matmul(sAB[:, 0, off:],
                                             KT[0:64, p, ksl],
                                             QT[0:64, p, qtr],
                                             start=True, stop=True)
                            nc.tensor.matmul(sAB[:, 1, off:],
                                             KT[64:128, p, ksl],
                                             QT[64:128, p, qtr],
                                             start=True, stop=True)
                            if prev is not None:
                                emit_ctx(*prev)
                            elif first and pending:
                                # normalize of the previous p emitted early
                                # inside this p's loop so its DVE/DMA chain
                                # overlaps a full kc loop of compute
                                first = False
                                pn = pending.pop(0)
                                normalize(*pn)
                            eAB = pE.tile([128, 2, 512], BF16, tag="E")
                            nc.scalar.activation(eAB[:, :, off:],
                                                 sAB[:, :, off:],
                                                 AF.Exp, scale=SCALE)
                            if diag:
                                nc.vector.tensor_mul(
                                    eAB[:, :, off:off + 128],
                                    eAB[:, :, off:off + 128],
                                    MASKT[:, None, :].to_broadcast(
                                        (128, 2, 128)))
                            prev = (kc, eAB, off)
                        emit_ctx(*prev)
                        dens = []
                        for i, cxt in enumerate((ctxA, ctxB)):
                            den = stB.tile([1, 512], F32, tag="den",
                                           name=f"den{i}", bufs=4)
                            nc.vector.tensor_copy(den[:], cxt[64:65, :])
                            dens.append((cxt, den))
                        pending.append((dens, p, qsl))
                        # previous qb's Wo emitted under this qb's first
                        # p-loop so it never waits on the normalize chain
                        if p == 0 and deferred_wo[0] is not None:
                            wo_block(deferred_wo[0])
                            deferred_wo[0] = None
                    deferred_wo[0] = qb
                while len(pending) > 1:
                    normalize(*pending.pop(0))
                normalize(*pending.pop(0), use_pe=True)
                wo_block(QB - 1)
                if dbg is not None:
                    nc.gpsimd.dma_start(dbg["dbg_ctx"][:], CTX[:])

        # ======== LN1 halves + FFN ========
        with (
            tc.tile_pool(name="pH1", bufs=1) as pH1,
            tc.tile_pool(name="pAOr", bufs=1) as pAOr,
            tc.tile_pool(name="pLN", bufs=1) as pLN,
            tc.tile_pool(name="stDE", bufs=1) as stDE,
            tc.tile_pool(name="psD", bufs=2, space="PSUM") as psD,
        ):
            H1 = [pH1.tile([128, DC, 512], BF16, tag=f"H1_{h}",
                           name=f"H1_{h}") for h in range(2)]

            def d_half(h, rso):
                aor = pAOr.tile([128, DC, 512], mybir.dt.float16, tag="AOr",
                                name=f"AOr{h}")
                nc.sync.dma_start(
                    aor[:], rso.rearrange("(dc p) t -> p dc t", p=128))
                xm = pAOr.tile([128, DC, 512], F32R, tag="XM",
                               name=f"XMt{h}")
                nc.sync.dma_start(xm[:], xTm_d[:, :, h * 512:(h + 1) * 512])
                nc.vector.tensor_add(xm[:], xm[:], aor[:])
                layer_norm(xm[:], lambda dc: H1[h][:, dc], g1t, e1t,
                           pLN, psD, stDE)
                if dbg is not None:
                    nc.gpsimd.dma_start(
                        dbg["dbg_h1"][:, :, h * 512:(h + 1) * 512],
                        H1[h][:])

            d_half(0, rs_out0)
            # second reduce-scatter (gpsimd blocks on collectives; it has no
            # other work left)
            nc.gpsimd.collective_compute(
                "ReduceScatter", mybir.AluOpType.add,
                replica_groups=[[0, 1], [2, 3], [4, 5], [6, 7]],
                ins=[rs_in1.opt()], outs=[rs_out1.opt()])

            # ======== FFN: W1 (resident weights) + W2 (streamed) ========
            with (
                tc.tile_pool(name="pFF", bufs=1) as pFF,
                tc.tile_pool(name="pO2", bufs=1) as pO2,
                tc.tile_pool(name="pW2q", bufs=2) as pW2q,
                tc.tile_pool(name="psF", bufs=2, space="PSUM") as psF,
                tc.tile_pool(name="psG", bufs=2, space="PSUM") as psG,
            ):
                def ffn_w1(th):
                    FFt = pFF.tile([128, FC, 512], BF16, tag="FF",
                                   name=f"FFt{th}")  # 32KB
                    for fc in range(FC):
                        pf = psF.tile([128, 512], F32, tag="ff")
                        for dc in range(DC):
                            nc.tensor.matmul(
                                pf[:], w1s[:, fc, dc], H1[th][:, dc],
                                start=(dc == 0), stop=(dc == DC - 1))
                        nc.scalar.activation(FFt[:, fc], pf[:], AF.Relu,
                                             bias=b1t[:, fc:fc + 1])
                    return FFt

                def ffn_w2(th, FFt):
                    O2 = pO2.tile([128, DC, 512], F32R, tag="O2",
                                  name=f"O2_{th}")
                    for dout in range(DC):
                        w2t = pW2q.tile([128, FC, 128], BF16, tag="w2")
                        nc.sync.dma_start(w2t[:], w2_d[dout])
                        po2 = psG.tile([128, 512], F32, tag="o2")
                        for fc in range(FC):
                            nc.tensor.matmul(po2[:], w2t[:, fc], FFt[:, fc],
                                             start=(fc == 0),
                                             stop=(fc == FC - 1))
                        # O2[dout] = (po2 + b2[dout]) + h1[dout]
                        nc.vector.scalar_tensor_tensor(
                            O2[:, dout], po2[:], b2t[:, dout:dout + 1],
                            H1[th][:, dout],
                            mybir.AluOpType.add, mybir.AluOpType.add)
                    if dbg is not None:
                        nc.gpsimd.dma_start(
                            dbg["dbg_o2"][:, :, th * 512:(th + 1) * 512],
                            O2[:])
                    def ot_dst(dc):
                        otc = pFF.tile([128, 512], F32, tag="ot", bufs=2,
                                       name="otc")
                        return otc[:]

                    def ot_post(dc, ap):
                        nc.sync.dma_start(
                            outT_d[:, dc, th * 512:(th + 1) * 512], ap)

                    layer_norm(O2[:], ot_dst, g2t, e2t, pLN, psD, stDE,
                               post=ot_post)

                ff0 = ffn_w1(0)
                d_half(1, rs_out1)
                ffn_w2(0, ff0)
                ff1 = ffn_w1(1)
                ffn_w2(1, ff1)


def _pack_inputs(x, Wq, Wk, Wv, Wo, ln1_g, ln1_b, W1, b1, W2, b2, ln2_g, ln2_b):
    """Build the 8 per-core input maps (all host-side numpy)."""
    import ml_dtypes
    bf = ml_dtypes.bfloat16
    f = np.float32
    x = np.asarray(x, f)
    Wq = np.asarray(Wq, f); Wk = np.asarray(Wk, f); Wv = np.asarray(Wv, f)
    Wo = np.asarray(Wo, f); W1 = np.asarray(W1, f); W2 = np.asarray(W2, f)
    in_maps = []
    w1p = np.ascontiguousarray(
        W1.reshape(DC, 128, FC, 128).transpose(1, 2, 0, 3)).astype(bf)
    w2p = np.ascontiguousarray(
        W2.reshape(FC, 128, DC, 128).transpose(2, 1, 0, 3)).astype(bf)
    b1s = np.ascontiguousarray(np.asarray(b1, f).reshape(FC, 128).T)
    b2s = np.ascontiguousarray(np.asarray(b2, f).reshape(DC, 128).T)
    g1s = np.ascontiguousarray(np.asarray(ln1_g, f).reshape(DC, 128).T)
    e1s = np.ascontiguousarray(np.asarray(ln1_b, f).reshape(DC, 128).T)
    g2s = np.ascontiguousarray(np.asarray(ln2_g, f).reshape(DC, 128).T)
    e2s = np.ascontiguousarray(np.asarray(ln2_b, f).reshape(DC, 128).T)
    kk = np.arange(128)[:, None]
    qq = np.arange(128)[None, :]
    maskt = np.ascontiguousarray((kk <= qq).astype(f)).astype(bf)

    for c in range(NCORES):
        b, j = c // 2, c % 2
        hb = j * HL
        xT = np.ascontiguousarray(
            x[b].T.reshape(DC, 128, S).transpose(1, 0, 2)).astype(bf)
        # owned token blocks: {j, j+2} of four 512-blocks
        xm = np.concatenate(
            [x[b, j * 512:(j + 1) * 512],
             x[b, (j + 2) * 512:(j + 3) * 512]]).T  # [D, TLOC]
        xTm = np.ascontiguousarray(
            xm.reshape(DC, 128, TLOC).transpose(1, 0, 2))
        wq = np.stack([np.concatenate([Wq[hb + 2 * p], Wq[hb + 2 * p + 1]], 1)
                       for p in range(NP)])  # [NP, D, 128]
        wq = np.ascontiguousarray(
            wq.reshape(NP, DC, 128, 128).transpose(0, 2, 1, 3)).astype(bf)
        wk = np.stack([np.concatenate([Wk[hb + 2 * p], Wk[hb + 2 * p + 1]], 1)
                       for p in range(NP)])
        wk = np.ascontiguousarray(
            wk.reshape(NP, DC, 128, 128).transpose(0, 2, 1, 3)).astype(bf)
        wv = np.concatenate([Wv[hb + i] for i in range(HL)], 1)  # [D, 512]
        wv = np.ascontiguousarray(
            wv.reshape(DC, 128, 512).transpose(1, 0, 2)).astype(bf)
        wo = Wo[j * 512:(j + 1) * 512]  # [512, D]
        wo = np.ascontiguousarray(
            wo.reshape(NP, 128, DC, 128).transpose(1, 0, 2, 3)).astype(bf)
        in_maps.append({
            "xT": xT, "xTmine": xTm, "wq": wq, "wk": wk, "wv": wv, "wo": wo,
            "w1": w1p, "w2": w2p, "b1s": b1s, "b2s": b2s,
            "g1s": g1s, "e1s": e1s, "g2s": g2s, "e2s": e2s, "maskt": maskt,
        })
    return in_maps


def get_compiled():
    global _COMPILED
    if _COMPILED is None:
        _COMPILED = _build()
    return _COMPILED


def kernel(x, Wq, Wk, Wv, Wo, ln1_g, ln1_b, W1, b1, W2, b2, ln2_g, ln2_b,
           _trace=False):
    nc = get_compiled()
    in_maps = _pack_inputs(x, Wq, Wk, Wv, Wo, ln1_g, ln1_b, W1, b1, W2, b2,
                           ln2_g, ln2_b)
    res = run_bass_kernel_spmd(nc, in_maps, core_ids=list(range(NCORES)),
                               trace=_trace)
    out = np.zeros((B, S, D), np.float32)
    for c in range(NCORES):
        b, j = c // 2, c % 2
        o = res.results[c]["outT"]  # [128, DC, TLOC]
        o = np.asarray(o, np.float32).transpose(1, 0, 2).reshape(D, TLOC)
        out[b, j * 512:(j + 1) * 512, :] = o[:, 0:512].T
        out[b, (j + 2) * 512:(j + 3) * 512, :] = o[:, 512:1024].T
    kernel.last_result = res
    return out
